# revision 1
# baseline (speedup 1.0000x reference)
"""Trainium2 Bass kernel for the BN-attention module (nn_Attention).

Full inputs -> full output. Sharding: 8 cores = (batch b in 0..3) x
(head-group g in 0..1, 4 heads each). Each core computes its batch's
4-head attention and a partial output projection; the host sums the two
head-group partials per batch and adds the projection BN bias.

Numerics: BN scales are folded into the weights on the host. QK^T and
the projections run as float32r (fp32 storage, FP22 multiply, fp32
accumulate; full PE rate at free dim >= 256). Softmax skips the
max-subtraction (logits are O(25) here, safe in fp32); exp(S^T) is
stored bf16, and both attn@V and the softmax denominators consume the
same bf16 values so their rounding largely cancels in the ratio.

Layout: attention is computed transposed, S^T = K^T Q with keys (m) on
partitions, so attn@V needs no transposes at all: V is produced
directly as vT[n,d] by the projection, and the denominators are
column sums done with ones-matmuls on the TensorE (plus one level of
bf16 pair-summing on the VectorE, which runs at 2x for bf16).
Normalization, BN-v bias, and ReLU fold into the PSUM eviction of xx.
"""

import numpy as np

import concourse.bacc as bacc
import concourse.mybir as mybir
import concourse.tile as tile
from concourse.bass_utils import run_bass_kernel_spmd

# Problem dims (hardcoded per the spec)
B, C, H, W = 4, 256, 48, 48
N = H * W            # 2304
KD, NH, AR = 32, 8, 4
D = AR * KD          # 128 value dims per head
NHKD = NH * KD       # 256
DH = NH * D          # 1024
EPS = 1e-5

NHG = 4              # heads per core
J = 256              # n-chunk width
NJ = N // J          # 9
MT = 128             # m-tile (key tile)
NMT = N // MT        # 18

F32 = mybir.dt.float32
F32R = mybir.dt.float32r
BF16 = mybir.dt.bfloat16
AF = mybir.ActivationFunctionType
OP = mybir.AluOpType

# chunks of 2304 by <=512 for the projection matmuls
CHUNKS_512 = [(off, min(512, N - off)) for off in range(0, N, 512)]

_CACHE = {}


def _build_program():
    nc = bacc.Bacc("TRN2", target_bir_lowering=False, debug=False)

    x_in = nc.dram_tensor("x_in", [C, N], F32R, kind="ExternalInput")
    wqt_d = nc.dram_tensor("wqt", [C, 128], F32R, kind="ExternalInput")
    wkt_d = nc.dram_tensor("wkt", [C, 128], F32R, kind="ExternalInput")
    wvt_d = nc.dram_tensor("wvt", [C, 512], F32R, kind="ExternalInput")
    wpt_d = nc.dram_tensor("wpt", [512, C], F32R, kind="ExternalInput")
    bq_d = nc.dram_tensor("bq", [128, 1], F32, kind="ExternalInput")
    bk_d = nc.dram_tensor("bk", [128, 1], F32, kind="ExternalInput")
    bv_d = nc.dram_tensor("bv", [512, 1], F32, kind="ExternalInput")
    consts_d = nc.dram_tensor("consts", [128, 128 + J], F32R, kind="ExternalInput")
    ones_bf_d = nc.dram_tensor("ones_bf", [128, 1], BF16, kind="ExternalInput")
    out_d = nc.dram_tensor("outp", [C, N], F32, kind="ExternalOutput")

    with tile.TileContext(nc) as tc:
        with nc.allow_low_precision(reason="float32r rounding is intentional"), \
             tc.tile_pool(name="const", bufs=1) as constp, \
             tc.tile_pool(name="qk", bufs=1) as qkp, \
             tc.tile_pool(name="vt", bufs=1) as vtp, \
             tc.tile_pool(name="pexp", bufs=1) as pexpp, \
             tc.tile_pool(name="rp", bufs=1) as rp, \
             tc.tile_pool(name="work", bufs=2) as workp:

            # ---------- constants / inputs ----------
            xf = []
            for c2 in range(2):
                t = constp.tile([128, N], F32R, name=f"xf{c2}", tag=f"xf{c2}")
                for off, w in CHUNKS_512:
                    nc.sync.dma_start(t[:, off:off + w],
                                      x_in.ap()[128 * c2:128 * (c2 + 1),
                                                off:off + w])
                xf.append(t)
            wqt, wkt, wvt = [], [], []
            for c2 in range(2):
                sl = slice(128 * c2, 128 * (c2 + 1))
                t = constp.tile([128, 128], F32R, name=f"wqt{c2}", tag=f"wqt{c2}")
                nc.sync.dma_start(t[:], wqt_d.ap()[sl, :])
                wqt.append(t)
                t = constp.tile([128, 128], F32R, name=f"wkt{c2}", tag=f"wkt{c2}")
                nc.sync.dma_start(t[:], wkt_d.ap()[sl, :])
                wkt.append(t)
                t = constp.tile([128, 512], F32R, name=f"wvt{c2}", tag=f"wvt{c2}")
                nc.sync.dma_start(t[:], wvt_d.ap()[sl, :])
                wvt.append(t)
            wpt = []
            for h in range(NHG):
                t = constp.tile([128, C], F32R, name=f"wpt{h}", tag=f"wpt{h}")
                nc.sync.dma_start(t[:], wpt_d.ap()[128 * h:128 * (h + 1), :])
                wpt.append(t)
            bq_t = constp.tile([128, 1], F32, name="bq_t", tag="bq_t")
            nc.sync.dma_start(bq_t[:], bq_d.ap())
            bk_t = constp.tile([128, 1], F32, name="bk_t", tag="bk_t")
            nc.sync.dma_start(bk_t[:], bk_d.ap())
            bv_t = []
            for h in range(NHG):
                t = constp.tile([128, 1], F32, name=f"bv{h}", tag=f"bv{h}")
                nc.sync.dma_start(t[:], bv_d.ap()[128 * h:128 * (h + 1), :])
                bv_t.append(t)
            # constants via DMA (memset can't produce float32r)
            consts_t = constp.tile([128, 128 + J], F32R, name="consts_t",
                                   tag="consts_t")
            nc.sync.dma_start(consts_t[:], consts_d.ap())
            ones_all = consts_t[:, 0:128]     # [128,128] of 1.0
            ones_bf = constp.tile([128, 1], BF16, name="ones_bf", tag="ones_bf")
            nc.sync.dma_start(ones_bf[:], ones_bf_d.ap())
            ones_t = consts_t[:, 0:1]         # [128,1] of 1.0
            zeros_row = consts_t[0:1, 128:128 + J]  # [1,J] of 0.0

            q_all = qkp.tile([128, N], F32R, name="q_all", tag="q_all")
            k_all = qkp.tile([128, N], F32R, name="k_all", tag="k_all")
            vt_all = vtp.tile([128, NMT * 512], BF16, name="vt_all", tag="vt_all")

            # ---------- phase 1: q/k/v projections ----------
            with tc.tile_pool(name="p1", bufs=4, space="PSUM") as p1:
                for off, w in CHUNKS_512:
                    ps = p1.tile([128, 512], F32, name="qproj", tag="p1")
                    for c2 in range(2):
                        nc.tensor.matmul(ps[:, :w], wqt[c2][:],
                                         xf[c2][:, off:off + w],
                                         start=(c2 == 0), stop=(c2 == 1))
                    nc.vector.tensor_scalar_add(q_all[:, off:off + w],
                                                ps[:, :w], bq_t[:])
                    ps = p1.tile([128, 512], F32, name="kproj", tag="p1")
                    for c2 in range(2):
                        nc.tensor.matmul(ps[:, :w], wkt[c2][:],
                                         xf[c2][:, off:off + w],
                                         start=(c2 == 0), stop=(c2 == 1))
                    nc.vector.tensor_scalar_add(k_all[:, off:off + w],
                                                ps[:, :w], bk_t[:])
                for nt in range(NMT):
                    ps = p1.tile([128, 512], F32, name="vproj", tag="p1")
                    for c2 in range(2):
                        nc.tensor.matmul(ps[:],
                                         xf[c2][:, 128 * nt:128 * (nt + 1)],
                                         wvt[c2][:],
                                         start=(c2 == 0), stop=(c2 == 1))
                    nc.vector.tensor_copy(vt_all[:, 512 * nt:512 * (nt + 1)],
                                          ps[:])

            # ---------- phase 2: attention + output projection ----------
            # n-chunks of width 512 (last 256); heads processed two at a
            # time (half-chunks). Per m-tile step, the two heads' S^T
            # matmuls land in the two banks of one [128,1024] psum tile
            # (alternating between two such tiles) and a single exp call
            # evicts both to a bf16 pexp tile. PV runs one step behind the
            # QKs so the PE never waits on ACT; consecutive pexp steps are
            # pair-summed on DVE (bf16 2x) into resident t-tiles, and the
            # denominator ones-matmuls + normalize + relu run in a finish
            # phase that uses two dedicated psum banks, overlapping the
            # next half-chunk's loop. The output projection needs all four
            # heads, so it runs after the second half-chunk.
            NP = NMT // 2
            JCHUNKS = [(0, 512), (512, 512), (1024, 512), (1536, 512),
                       (2048, 256)]
            with tc.tile_pool(name="stp", bufs=1, space="PSUM") as stp, \
                 tc.tile_pool(name="xxp", bufs=1, space="PSUM") as xxp, \
                 tc.tile_pool(name="finp", bufs=1, space="PSUM") as finp:
                for joff, JW in JCHUNKS:
                    r_ts = [None] * NHG
                    for ha, hb in ((0, 1), (2, 3)):
                        xx = {}
                        for h in (ha, hb):
                            xx[h] = xxp.tile([128, 512], F32, name=f"xx{h % 2}",
                                             tag=f"xx{h % 2}")[:, 0:JW]
                        pexp = [None] * NMT
                        tsum = [None] * NP

                        def emit_qk_exp(mt):
                            # the two heads' matmuls run concurrently
                            # (different row groups), so they must land in
                            # different psum banks: halves at 0 and 512.
                            st = stp.tile([128, 1024], F32, name="st",
                                          tag=f"st{mt % 2}")
                            moff = 128 * mt
                            for i, h in enumerate((ha, hb)):
                                nc.tensor.matmul(
                                    st[:, 512 * i:512 * i + JW],
                                    k_all[32 * h:32 * (h + 1), moff:moff + 128],
                                    q_all[32 * h:32 * (h + 1), joff:joff + JW],
                                    start=True, stop=True,
                                    tile_position=(32 * h, 0))
                            pe = pexpp.tile([128, 1024], BF16, name="pe",
                                            tag="pe", bufs=3)
                            if JW == 512:
                                nc.scalar.activation(pe[:, 0:1024],
                                                     st[:, 0:1024], AF.Exp)
                            else:
                                st_v = st.rearrange("p (a b) -> p a b",
                                                    b=512)[:, :, 0:JW]
                                pe_v = pe.rearrange("p (a b) -> p a b",
                                                    b=512)[:, :, 0:JW]
                                nc.scalar.activation(pe_v, st_v, AF.Exp)
                            pexp[mt] = pe

                        def emit_pv(mt):
                            pe = pexp[mt]
                            for i, h in enumerate((ha, hb)):
                                nc.tensor.matmul(
                                    xx[h],
                                    vt_all[:, 512 * mt + 128 * h:
                                           512 * mt + 128 * (h + 1)],
                                    pe[:, 512 * i:512 * i + JW],
                                    start=(mt == 0), stop=(mt == NMT - 1))

                        for mt in range(NMT):
                            emit_qk_exp(mt)
                            if mt > 0:
                                emit_pv(mt - 1)
                            if mt % 2 == 1:
                                k = mt // 2
                                t = workp.tile([128, 1024], BF16,
                                               name=f"t{k}", tag=f"t{k}",
                                               bufs=1)
                                nc.vector.tensor_tensor(
                                    t[:, 0:512 + JW],
                                    pexp[mt - 1][:, 0:512 + JW],
                                    pexp[mt][:, 0:512 + JW], OP.add)
                                tsum[k] = t
                        emit_pv(NMT - 1)

                        # finish: denominators, normalize, bias+relu
                        for i, h in enumerate((ha, hb)):
                            fslot = f"f{i}"
                            sums_h = finp.tile([1, 512], F32, name="sums_h",
                                               tag=fslot)
                            for k in range(NP):
                                nc.tensor.matmul(
                                    sums_h[:, 0:JW], ones_bf[:],
                                    tsum[k][:, 512 * i:512 * i + JW],
                                    start=(k == 0), stop=(k == NP - 1))
                            s_row = workp.tile([1, 512], F32R, name="s_row",
                                               tag="s_row")
                            nc.vector.tensor_copy(s_row[:, 0:JW],
                                                  sums_h[:, 0:JW])
                            s_bc = finp.tile([128, 512], F32, name="s_bc",
                                             tag=fslot)
                            nc.tensor.matmul(s_bc[:, 0:JW], ones_all[0:1, :],
                                             s_row[:, 0:JW],
                                             start=True, stop=True)
                            inv_s = workp.tile([128, 512], F32, name="inv_s",
                                               tag="inv_s")
                            nc.vector.reciprocal_approx_fast(inv_s[:, 0:JW],
                                                             s_bc[:, 0:JW])
                            t_h = workp.tile([128, 512], F32, name="t_h",
                                             tag="t_h")
                            nc.vector.tensor_tensor(t_h[:, 0:JW], xx[h],
                                                    inv_s[:, 0:JW], OP.mult)
                            r_h = rp.tile([128, 512], F32R, name=f"r{h}",
                                          tag=f"r{h}")
                            nc.scalar.activation(r_h[:, 0:JW], t_h[:, 0:JW],
                                                 AF.Relu, bias=bv_t[h][:])
                            r_ts[h] = r_h

                    # output projection over all four heads
                    for ct in range(2):
                        op_ps = finp.tile([128, 512], F32, name="op_ps",
                                          tag=f"f{ct}")
                        for h in range(NHG):
                            nc.tensor.matmul(
                                op_ps[:, 0:JW],
                                wpt[h][:, 128 * ct:128 * (ct + 1)],
                                r_ts[h][:, 0:JW],
                                start=(h == 0), stop=(h == NHG - 1))
                        o_sb = workp.tile([128, 512], F32, name="o_sb",
                                          tag="o_sb")
                        nc.vector.tensor_copy(o_sb[:, 0:JW], op_ps[:, 0:JW])
                        nc.sync.dma_start(
                            out_d.ap()[128 * ct:128 * (ct + 1), joff:joff + JW],
                            o_sb[:, 0:JW])
    nc.compile()
    return nc


def _prep_inputs(x, wq, gq, bq, wk, gk, bk, wv, gv, bv, wp, gp, bp):
    """Fold BN scales into weights; build the 8 per-core input maps."""
    rs = np.float32(1.0 / np.sqrt(np.float32(1.0) + np.float32(EPS)))
    sq = (gq * rs).astype(np.float32)
    sk = (gk * rs).astype(np.float32)
    sv = (gv * rs).astype(np.float32)
    sp = (gp * rs).astype(np.float32)
    wq_f = (wq * sq[:, None]).astype(np.float32)
    wk_f = (wk * sk[:, None]).astype(np.float32)
    wv_f = (wv * sv[:, None]).astype(np.float32)
    wp_f = (wp * sp[:, None]).astype(np.float32)

    xf = np.ascontiguousarray(x.reshape(B, C, N).astype(np.float32))
    consts = np.zeros((128, 128 + J), dtype=np.float32)
    consts[:, 0:128] = 1.0
    import ml_dtypes
    ones_bf = np.ones((128, 1), dtype=ml_dtypes.bfloat16)
    in_maps = []
    for core in range(8):
        b, g = core // 2, core % 2
        qs = slice(128 * g, 128 * (g + 1))       # q/k rows for this head group
        vs = slice(512 * g, 512 * (g + 1))       # v rows / p cols for this group
        in_maps.append({
            "x_in": xf[b],
            "wqt": np.ascontiguousarray(wq_f[qs, :].T),
            "wkt": np.ascontiguousarray(wk_f[qs, :].T),
            "wvt": np.ascontiguousarray(wv_f[vs, :].T),
            "wpt": np.ascontiguousarray(wp_f[:, vs].T),
            "bq": np.ascontiguousarray(bq[qs].astype(np.float32)[:, None]),
            "bk": np.ascontiguousarray(bk[qs].astype(np.float32)[:, None]),
            "bv": np.ascontiguousarray(bv[vs].astype(np.float32)[:, None]),
            "consts": consts,
            "ones_bf": ones_bf,
        })
    return in_maps


def kernel(**inputs):
    if "nc" not in _CACHE:
        _CACHE["nc"] = _build_program()
    nc = _CACHE["nc"]

    in_maps = _prep_inputs(**{k: np.asarray(v) for k, v in inputs.items()})
    res = run_bass_kernel_spmd(nc, in_maps, list(range(8)))
    _CACHE["last_results"] = res

    bp = np.asarray(inputs["bp"]).astype(np.float32)
    out = np.empty((B, C, H, W), dtype=np.float32)
    for b in range(B):
        acc = res.results[2 * b]["outp"] + res.results[2 * b + 1]["outp"]
        acc = acc + bp[:, None]
        out[b] = acc.reshape(C, H, W)
    return out



# revision 3
# speedup vs baseline: 1.2176x; 1.2176x over previous
"""Trainium2 Bass kernel for the BN-attention module (nn_Attention).

Full inputs -> full output. Sharding: 8 cores = (batch b in 0..3) x
(head-group g in 0..1, 4 heads each). Each core computes its batch's
4-head attention and a partial output projection; the host sums the two
head-group partials per batch and adds the projection BN bias.

Numerics: BN scales are folded into the weights on the host. The Q/K
path (x, wq, wk, q, k) runs in fp16 (11-bit mantissa; logit error
~4e-3 absolute). The V/P paths run 16-bit as well: vT and exp(S^T) are
bf16 (exp needs bf16 range, up to e^~25); relu output and wp are fp16.
Softmax skips the max-subtraction (logits are O(25), safe in fp32
accumulation); both attn@V and the softmax denominators consume the
same bf16 exp values so their rounding largely cancels in the ratio.

Layout: attention is computed transposed, S^T = K^T Q with keys (m) on
partitions, so attn@V needs no transposes at all: V is produced
directly as vT[n,d] by the projection. Denominators are column sums:
exp tiles are pair-summed on the VectorE (bf16 2x) into a 3-level
tree (9 -> 4+t8 -> 2+t8), then 3 ones-matmuls accumulate the column
sums on the TensorE. The finish phase (reciprocal, broadcast,
normalize, bias+relu) runs chunk-wide: one reciprocal per [128, 1024]
chunk, relu+bias as a dual-op tensor_scalar on the VectorE (fp16 out),
keeping the ScalarE exp-only.
"""

import numpy as np

import concourse.bacc as bacc
import concourse.mybir as mybir
import concourse.tile as tile
from concourse.bass_utils import run_bass_kernel_spmd

# Problem dims (hardcoded per the spec)
B, C, H, W = 4, 256, 48, 48
N = H * W            # 2304
KD, NH, AR = 32, 8, 4
D = AR * KD          # 128 value dims per head
NHKD = NH * KD       # 256
DH = NH * D          # 1024
EPS = 1e-5

NHG = 4              # heads per core
J = 256              # (legacy) consts tile width
MT = 128             # m-tile (key tile)
NMT = N // MT        # 18

F32 = mybir.dt.float32
F32R = mybir.dt.float32r
BF16 = mybir.dt.bfloat16
FP16 = mybir.dt.float16
AF = mybir.ActivationFunctionType
OP = mybir.AluOpType

# chunks of 2304 by <=512 for the projection matmuls
CHUNKS_512 = [(off, min(512, N - off)) for off in range(0, N, 512)]

_CACHE = {}


def _build_program():
    nc = bacc.Bacc("TRN2", target_bir_lowering=False, debug=False)

    x_in = nc.dram_tensor("x_in", [C, N], FP16, kind="ExternalInput")
    wqt_d = nc.dram_tensor("wqt", [C, 128], FP16, kind="ExternalInput")
    wkt_d = nc.dram_tensor("wkt", [C, 128], FP16, kind="ExternalInput")
    wvt_d = nc.dram_tensor("wvt", [C, 512], FP16, kind="ExternalInput")
    wpt_d = nc.dram_tensor("wpt", [512, C], FP16, kind="ExternalInput")
    bq_d = nc.dram_tensor("bq", [128, 1], F32, kind="ExternalInput")
    bk_d = nc.dram_tensor("bk", [128, 1], F32, kind="ExternalInput")
    bv_d = nc.dram_tensor("bv", [512, 1], F32, kind="ExternalInput")
    consts_d = nc.dram_tensor("consts", [128, 128 + J], F32R, kind="ExternalInput")
    ones_bf_d = nc.dram_tensor("ones_bf", [128, 1], BF16, kind="ExternalInput")
    out_d = nc.dram_tensor("outp", [C, N], F32, kind="ExternalOutput")

    with tile.TileContext(nc) as tc:
        with nc.allow_low_precision(reason="16-bit matmul rounding is intentional"), \
             tc.tile_pool(name="const", bufs=1) as constp, \
             tc.tile_pool(name="qk", bufs=1) as qkp, \
             tc.tile_pool(name="vt", bufs=1) as vtp, \
             tc.tile_pool(name="pexp", bufs=1) as pexpp, \
             tc.tile_pool(name="rp", bufs=1) as rp, \
             tc.tile_pool(name="work", bufs=2) as workp:

            # ---------- constants / inputs ----------
            xf = []
            for c2 in range(2):
                t = constp.tile([128, N], FP16, name=f"xf{c2}", tag=f"xf{c2}")
                for off, w in CHUNKS_512:
                    nc.sync.dma_start(t[:, off:off + w],
                                      x_in.ap()[128 * c2:128 * (c2 + 1),
                                                off:off + w])
                xf.append(t)
            wqt, wkt, wvt = [], [], []
            for c2 in range(2):
                sl = slice(128 * c2, 128 * (c2 + 1))
                t = constp.tile([128, 128], FP16, name=f"wqt{c2}", tag=f"wqt{c2}")
                nc.sync.dma_start(t[:], wqt_d.ap()[sl, :])
                wqt.append(t)
                t = constp.tile([128, 128], FP16, name=f"wkt{c2}", tag=f"wkt{c2}")
                nc.sync.dma_start(t[:], wkt_d.ap()[sl, :])
                wkt.append(t)
                t = constp.tile([128, 512], FP16, name=f"wvt{c2}", tag=f"wvt{c2}")
                nc.sync.dma_start(t[:], wvt_d.ap()[sl, :])
                wvt.append(t)
            wpt = []
            for h in range(NHG):
                t = constp.tile([128, C], FP16, name=f"wpt{h}", tag=f"wpt{h}")
                nc.sync.dma_start(t[:], wpt_d.ap()[128 * h:128 * (h + 1), :])
                wpt.append(t)
            bq_t = constp.tile([128, 1], F32, name="bq_t", tag="bq_t")
            nc.sync.dma_start(bq_t[:], bq_d.ap())
            bk_t = constp.tile([128, 1], F32, name="bk_t", tag="bk_t")
            nc.sync.dma_start(bk_t[:], bk_d.ap())
            bv_t = []
            for h in range(NHG):
                t = constp.tile([128, 1], F32, name=f"bv{h}", tag=f"bv{h}")
                nc.sync.dma_start(t[:], bv_d.ap()[128 * h:128 * (h + 1), :])
                bv_t.append(t)
            # constants via DMA (memset can't produce float32r)
            consts_t = constp.tile([128, 128 + J], F32R, name="consts_t",
                                   tag="consts_t")
            nc.sync.dma_start(consts_t[:], consts_d.ap())
            ones_all = consts_t[:, 0:128]     # [128,128] of 1.0
            ones_bf = constp.tile([128, 1], BF16, name="ones_bf", tag="ones_bf")
            nc.sync.dma_start(ones_bf[:], ones_bf_d.ap())

            q_all = qkp.tile([128, N], FP16, name="q_all", tag="q_all")
            k_all = qkp.tile([128, N], FP16, name="k_all", tag="k_all")
            vt_all = vtp.tile([128, NMT * 512], BF16, name="vt_all", tag="vt_all")

            # ---------- phase 1: q/k/v projections ----------
            with tc.tile_pool(name="p1", bufs=4, space="PSUM") as p1:
                for off, w in CHUNKS_512:
                    ps = p1.tile([128, 512], F32, name="qproj", tag="p1")
                    for c2 in range(2):
                        nc.tensor.matmul(ps[:, :w], wqt[c2][:],
                                         xf[c2][:, off:off + w],
                                         start=(c2 == 0), stop=(c2 == 1))
                    nc.vector.tensor_scalar_add(q_all[:, off:off + w],
                                                ps[:, :w], bq_t[:])
                    ps = p1.tile([128, 512], F32, name="kproj", tag="p1")
                    for c2 in range(2):
                        nc.tensor.matmul(ps[:, :w], wkt[c2][:],
                                         xf[c2][:, off:off + w],
                                         start=(c2 == 0), stop=(c2 == 1))
                    nc.vector.tensor_scalar_add(k_all[:, off:off + w],
                                                ps[:, :w], bk_t[:])
                for nt in range(NMT):
                    ps = p1.tile([128, 512], F32, name="vproj", tag="p1")
                    for c2 in range(2):
                        nc.tensor.matmul(ps[:],
                                         xf[c2][:, 128 * nt:128 * (nt + 1)],
                                         wvt[c2][:],
                                         start=(c2 == 0), stop=(c2 == 1))
                    nc.vector.tensor_copy(vt_all[:, 512 * nt:512 * (nt + 1)],
                                          ps[:])

            # ---------- phase 2: attention + output projection ----------
            # n-chunks of width 512 (last 256); heads processed two at a
            # time (half-chunks). Per m-tile step, the two heads' S^T
            # matmuls land in the two banks of one [128,1024] psum tile
            # (alternating between two such tiles) and a single exp call
            # evicts both to a bf16 pexp tile. PV runs one step behind the
            # QKs so the PE never waits on ACT; consecutive pexp steps are
            # pair-summed on DVE (bf16 2x) into resident t-tiles, which a
            # 2-more-level DVE tree folds to 3 tiles so the denominator
            # needs only 3 ones-matmuls. The finish phase (reciprocal,
            # broadcast, normalize, fp16 relu+bias on DVE) runs chunk-wide
            # in two dedicated psum banks, overlapping the next
            # half-chunk's loop. The output projection needs all four
            # heads, so it runs after the second half-chunk.
            NP = NMT // 2
            JCHUNKS = [(0, 512), (512, 512), (1024, 512), (1536, 512),
                       (2048, 256)]
            with tc.tile_pool(name="stp", bufs=1, space="PSUM") as stp, \
                 tc.tile_pool(name="xxp", bufs=1, space="PSUM") as xxp, \
                 tc.tile_pool(name="finp", bufs=1, space="PSUM") as finp:
                for joff, JW in JCHUNKS:
                    SW = 512 + JW      # combined two-head column width
                    r_ts = [None] * NHG
                    for ha, hb in ((0, 1), (2, 3)):
                        pi = ha // 2   # pair index 0/1
                        xx = {}
                        for h in (ha, hb):
                            xx[h] = xxp.tile([128, 512], F32, name=f"xx{h % 2}",
                                             tag=f"xx{h % 2}")[:, 0:JW]
                        pexp = [None] * NMT
                        tsum = [None] * NP
                        usum = [None] * 4
                        wsum = [None] * 2

                        def emit_qk_exp(mt):
                            # the two heads' matmuls run concurrently
                            # (different row groups), so they must land in
                            # different psum banks: halves at 0 and 512.
                            st = stp.tile([128, 1024], F32, name="st",
                                          tag=f"st{mt % 2}")
                            moff = 128 * mt
                            for i, h in enumerate((ha, hb)):
                                nc.tensor.matmul(
                                    st[:, 512 * i:512 * i + JW],
                                    k_all[32 * h:32 * (h + 1), moff:moff + 128],
                                    q_all[32 * h:32 * (h + 1), joff:joff + JW],
                                    start=True, stop=True,
                                    tile_position=(32 * h, 0))
                            pe = pexpp.tile([128, 1024], BF16, name="pe",
                                            tag="pe", bufs=3)
                            if JW == 512:
                                nc.scalar.activation(pe[:, 0:1024],
                                                     st[:, 0:1024], AF.Exp)
                            else:
                                st_v = st.rearrange("p (a b) -> p a b",
                                                    b=512)[:, :, 0:JW]
                                pe_v = pe.rearrange("p (a b) -> p a b",
                                                    b=512)[:, :, 0:JW]
                                nc.scalar.activation(pe_v, st_v, AF.Exp)
                            pexp[mt] = pe

                        def emit_pv(mt):
                            pe = pexp[mt]
                            for i, h in enumerate((ha, hb)):
                                nc.tensor.matmul(
                                    xx[h],
                                    vt_all[:, 512 * mt + 128 * h:
                                           512 * mt + 128 * (h + 1)],
                                    pe[:, 512 * i:512 * i + JW],
                                    start=(mt == 0), stop=(mt == NMT - 1))

                        def emit_tree(k):
                            # level-2/3 folds as soon as inputs are ready
                            if k % 2 == 1 and k < 8:
                                u = k // 2
                                t = workp.tile([128, 1024], BF16,
                                               name=f"u{u}", tag=f"u{u}",
                                               bufs=1)
                                nc.vector.tensor_tensor(
                                    t[:, 0:SW], tsum[k - 1][:, 0:SW],
                                    tsum[k][:, 0:SW], OP.add)
                                usum[u] = t
                                if u % 2 == 1:
                                    w_ = u // 2
                                    t2 = workp.tile([128, 1024], BF16,
                                                    name=f"w{w_}",
                                                    tag=f"w{w_}", bufs=1)
                                    nc.vector.tensor_tensor(
                                        t2[:, 0:SW], usum[u - 1][:, 0:SW],
                                        usum[u][:, 0:SW], OP.add)
                                    wsum[w_] = t2

                        for mt in range(NMT):
                            emit_qk_exp(mt)
                            if mt > 0:
                                emit_pv(mt - 1)
                            if mt % 2 == 1:
                                k = mt // 2
                                t = workp.tile([128, 1024], BF16,
                                               name=f"t{k}", tag=f"t{k}",
                                               bufs=1)
                                nc.vector.tensor_tensor(
                                    t[:, 0:SW],
                                    pexp[mt - 1][:, 0:SW],
                                    pexp[mt][:, 0:SW], OP.add)
                                tsum[k] = t
                                emit_tree(k)
                        emit_pv(NMT - 1)

                        # finish: denominators, normalize, bias+relu.
                        # 3 ones-matmuls (w0, w1, t8) accumulate the
                        # column sums per head.
                        dens = [wsum[0], wsum[1], tsum[8]]
                        for i, h in enumerate((ha, hb)):
                            fslot = f"f{i}"
                            sums_h = finp.tile([1, 512], F32, name="sums_h",
                                               tag=fslot)
                            for k, dt_ in enumerate(dens):
                                nc.tensor.matmul(
                                    sums_h[:, 0:JW], ones_bf[:],
                                    dt_[:, 512 * i:512 * i + JW],
                                    start=(k == 0),
                                    stop=(k == len(dens) - 1))
                            s_row = workp.tile([1, 512], F32R, name="s_row",
                                               tag="s_row")
                            nc.vector.tensor_copy(s_row[:, 0:JW],
                                                  sums_h[:, 0:JW])
                            s_bc = finp.tile([128, 512], F32, name="s_bc",
                                             tag=fslot)
                            nc.tensor.matmul(s_bc[:, 0:JW], ones_all[0:1, :],
                                             s_row[:, 0:JW],
                                             start=True, stop=True)
                            inv_s = workp.tile([128, 512], F32, name="inv_s",
                                               tag="inv_s")
                            nc.vector.reciprocal_approx_fast(inv_s[:, 0:JW],
                                                             s_bc[:, 0:JW])
                            t_h = workp.tile([128, 512], F32, name="t_h",
                                             tag="t_h")
                            nc.vector.tensor_tensor(t_h[:, 0:JW], xx[h],
                                                    inv_s[:, 0:JW], OP.mult)
                            r_h = rp.tile([128, 512], FP16, name=f"r{h}",
                                          tag=f"r{h}")
                            nc.vector.tensor_scalar(
                                r_h[:, 0:JW], t_h[:, 0:JW], bv_t[h][:], 0.0,
                                OP.add, OP.max)
                            r_ts[h] = r_h

                    # output projection over all four heads
                    for ct in range(2):
                        op_ps = finp.tile([128, 512], F32, name="op_ps",
                                          tag=f"f{ct}")
                        for h in range(NHG):
                            nc.tensor.matmul(
                                op_ps[:, 0:JW],
                                wpt[h][:, 128 * ct:128 * (ct + 1)],
                                r_ts[h][:, 0:JW],
                                start=(h == 0), stop=(h == NHG - 1))
                        o_sb = workp.tile([128, 512], F32, name="o_sb",
                                          tag="o_sb")
                        nc.vector.tensor_copy(o_sb[:, 0:JW], op_ps[:, 0:JW])
                        nc.sync.dma_start(
                            out_d.ap()[128 * ct:128 * (ct + 1), joff:joff + JW],
                            o_sb[:, 0:JW])
    nc.compile()
    return nc


def _prep_inputs(x, wq, gq, bq, wk, gk, bk, wv, gv, bv, wp, gp, bp):
    """Fold BN scales into weights; build the 8 per-core input maps."""
    rs = np.float32(1.0 / np.sqrt(np.float32(1.0) + np.float32(EPS)))
    sq = (gq * rs).astype(np.float32)
    sk = (gk * rs).astype(np.float32)
    sv = (gv * rs).astype(np.float32)
    sp = (gp * rs).astype(np.float32)
    wq_f = (wq * sq[:, None]).astype(np.float16)
    wk_f = (wk * sk[:, None]).astype(np.float16)
    wv_f = (wv * sv[:, None]).astype(np.float16)
    wp_f = (wp * sp[:, None]).astype(np.float16)

    xf = np.ascontiguousarray(x.reshape(B, C, N).astype(np.float16))
    consts = np.zeros((128, 128 + J), dtype=np.float32)
    consts[:, 0:128] = 1.0
    import ml_dtypes
    ones_bf = np.ones((128, 1), dtype=ml_dtypes.bfloat16)
    in_maps = []
    for core in range(8):
        b, g = core // 2, core % 2
        qs = slice(128 * g, 128 * (g + 1))       # q/k rows for this head group
        vs = slice(512 * g, 512 * (g + 1))       # v rows / p cols for this group
        in_maps.append({
            "x_in": xf[b],
            "wqt": np.ascontiguousarray(wq_f[qs, :].T),
            "wkt": np.ascontiguousarray(wk_f[qs, :].T),
            "wvt": np.ascontiguousarray(wv_f[vs, :].T),
            "wpt": np.ascontiguousarray(wp_f[:, vs].T),
            "bq": np.ascontiguousarray(bq[qs].astype(np.float32)[:, None]),
            "bk": np.ascontiguousarray(bk[qs].astype(np.float32)[:, None]),
            "bv": np.ascontiguousarray(bv[vs].astype(np.float32)[:, None]),
            "consts": consts,
            "ones_bf": ones_bf,
        })
    return in_maps


def kernel(**inputs):
    if "nc" not in _CACHE:
        _CACHE["nc"] = _build_program()
    nc = _CACHE["nc"]

    in_maps = _prep_inputs(**{k: np.asarray(v) for k, v in inputs.items()})
    res = run_bass_kernel_spmd(nc, in_maps, list(range(8)))
    _CACHE["last_results"] = res

    bp = np.asarray(inputs["bp"]).astype(np.float32)
    out = np.empty((B, C, H, W), dtype=np.float32)
    for b in range(B):
        acc = res.results[2 * b]["outp"] + res.results[2 * b + 1]["outp"]
        acc = acc + bp[:, None]
        out[b] = acc.reshape(C, H, W)
    return out


# revision 5
# speedup vs baseline: 1.2420x; 1.0200x over previous
"""Trainium2 Bass kernel for the BN-attention module (nn_Attention).

Full inputs -> full output. Sharding: 8 cores = (batch b in 0..3) x
(head-group g in 0..1, 4 heads each). Each core computes its batch's
4-head attention and a partial output projection; the host sums the two
head-group partials per batch and adds the projection BN bias.

Numerics: BN scales are folded into the weights on the host. The Q/K
path (x, wq, wk, q, k) runs in fp16 (11-bit mantissa; logit error
~4e-3 absolute). The V/P paths run 16-bit as well: vT and exp(S^T) are
bf16 (exp needs bf16 range, up to e^~25); relu output and wp are fp16.
Softmax skips the max-subtraction (logits are O(25), safe in fp32
accumulation); both attn@V and the softmax denominators consume the
same bf16 exp values so their rounding largely cancels in the ratio.

Layout: attention is computed transposed, S^T = K^T Q with keys (m) on
partitions, so attn@V needs no transposes at all: V is produced
directly as vT[n,d] by the projection. Denominators are column sums:
exp tiles are pair-summed on the VectorE (bf16 2x) into a 3-level
tree (9 -> 4+t8 -> 2+t8), then 3 ones-matmuls accumulate the column
sums on the TensorE. The finish phase (reciprocal, broadcast,
normalize, bias+relu) runs chunk-wide: one reciprocal per [128, 1024]
chunk, relu+bias as a dual-op tensor_scalar on the VectorE (fp16 out),
keeping the ScalarE exp-only.
"""

import numpy as np

import concourse.bacc as bacc
import concourse.mybir as mybir
import concourse.tile as tile
from concourse.bass_utils import run_bass_kernel_spmd

# Problem dims (hardcoded per the spec)
B, C, H, W = 4, 256, 48, 48
N = H * W            # 2304
KD, NH, AR = 32, 8, 4
D = AR * KD          # 128 value dims per head
NHKD = NH * KD       # 256
DH = NH * D          # 1024
EPS = 1e-5

NHG = 4              # heads per core
J = 256              # (legacy) consts tile width
MT = 128             # m-tile (key tile)
NMT = N // MT        # 18

F32 = mybir.dt.float32
F32R = mybir.dt.float32r
BF16 = mybir.dt.bfloat16
FP16 = mybir.dt.float16
AF = mybir.ActivationFunctionType
OP = mybir.AluOpType

# chunks of 2304 by <=512 for the projection matmuls
CHUNKS_512 = [(off, min(512, N - off)) for off in range(0, N, 512)]

_CACHE = {}


def _build_program():
    nc = bacc.Bacc("TRN2", target_bir_lowering=False, debug=False)

    x_in = nc.dram_tensor("x_in", [C, N], FP16, kind="ExternalInput")
    wqt_d = nc.dram_tensor("wqt", [C, 128], FP16, kind="ExternalInput")
    wkt_d = nc.dram_tensor("wkt", [C, 128], FP16, kind="ExternalInput")
    wvt_d = nc.dram_tensor("wvt", [C, 512], FP16, kind="ExternalInput")
    wpt_d = nc.dram_tensor("wpt", [512, C], FP16, kind="ExternalInput")
    bq_d = nc.dram_tensor("bq", [128, 1], F32, kind="ExternalInput")
    bk_d = nc.dram_tensor("bk", [128, 1], F32, kind="ExternalInput")
    bv_d = nc.dram_tensor("bv", [512, 1], F32, kind="ExternalInput")
    consts_d = nc.dram_tensor("consts", [128, 128 + J], F32R, kind="ExternalInput")
    ones_bf_d = nc.dram_tensor("ones_bf", [128, 1], BF16, kind="ExternalInput")
    out_d = nc.dram_tensor("outp", [C, N], F32, kind="ExternalOutput")

    with tile.TileContext(nc) as tc:
        with nc.allow_low_precision(reason="16-bit matmul rounding is intentional"), \
             tc.tile_pool(name="const", bufs=1) as constp, \
             tc.tile_pool(name="qk", bufs=1) as qkp, \
             tc.tile_pool(name="vt", bufs=1) as vtp, \
             tc.tile_pool(name="pexp", bufs=1) as pexpp, \
             tc.tile_pool(name="rp", bufs=1) as rp, \
             tc.tile_pool(name="work", bufs=2) as workp:

            # ---------- constants / inputs ----------
            # DMA order matters: the q/k weights + biases + x chunk 0 go
            # first so phase 1 starts as early as possible.
            wqt, wkt, wvt = [], [], []
            for c2 in range(2):
                sl = slice(128 * c2, 128 * (c2 + 1))
                t = constp.tile([128, 128], FP16, name=f"wqt{c2}", tag=f"wqt{c2}")
                nc.sync.dma_start(t[:], wqt_d.ap()[sl, :])
                wqt.append(t)
                t = constp.tile([128, 128], FP16, name=f"wkt{c2}", tag=f"wkt{c2}")
                nc.sync.dma_start(t[:], wkt_d.ap()[sl, :])
                wkt.append(t)
            bq_t = constp.tile([128, 1], F32, name="bq_t", tag="bq_t")
            nc.sync.dma_start(bq_t[:], bq_d.ap())
            bk_t = constp.tile([128, 1], F32, name="bk_t", tag="bk_t")
            nc.sync.dma_start(bk_t[:], bk_d.ap())
            xf = [constp.tile([128, N], FP16, name=f"xf{c2}", tag=f"xf{c2}")
                  for c2 in range(2)]
            for off, w in CHUNKS_512:
                for c2 in range(2):
                    nc.sync.dma_start(xf[c2][:, off:off + w],
                                      x_in.ap()[128 * c2:128 * (c2 + 1),
                                                off:off + w])
                if off == 0:
                    for c2 in range(2):
                        t = constp.tile([128, 512], FP16, name=f"wvt{c2}",
                                        tag=f"wvt{c2}")
                        nc.sync.dma_start(t[:], wvt_d.ap()[
                            128 * c2:128 * (c2 + 1), :])
                        wvt.append(t)
            wpt = []
            for h in range(NHG):
                t = constp.tile([128, C], FP16, name=f"wpt{h}", tag=f"wpt{h}")
                nc.sync.dma_start(t[:], wpt_d.ap()[128 * h:128 * (h + 1), :])
                wpt.append(t)
            bv_t = []
            for h in range(NHG):
                t = constp.tile([128, 1], F32, name=f"bv{h}", tag=f"bv{h}")
                nc.sync.dma_start(t[:], bv_d.ap()[128 * h:128 * (h + 1), :])
                bv_t.append(t)
            # constants via DMA (memset can't produce float32r)
            consts_t = constp.tile([128, 128 + J], F32R, name="consts_t",
                                   tag="consts_t")
            nc.sync.dma_start(consts_t[:], consts_d.ap())
            ones_all = consts_t[:, 0:128]     # [128,128] of 1.0
            ones_bf = constp.tile([128, 1], BF16, name="ones_bf", tag="ones_bf")
            nc.sync.dma_start(ones_bf[:], ones_bf_d.ap())

            q_all = qkp.tile([128, N], FP16, name="q_all", tag="q_all")
            k_all = qkp.tile([128, N], FP16, name="k_all", tag="k_all")
            vt_all = vtp.tile([128, NMT * 512], BF16, name="vt_all", tag="vt_all")

            # ---------- phase 1: q/k/v projections ----------
            with tc.tile_pool(name="p1", bufs=4, space="PSUM") as p1:
                for off, w in CHUNKS_512:
                    ps = p1.tile([128, 512], F32, name="qproj", tag="p1")
                    for c2 in range(2):
                        nc.tensor.matmul(ps[:, :w], wqt[c2][:],
                                         xf[c2][:, off:off + w],
                                         start=(c2 == 0), stop=(c2 == 1))
                    nc.vector.tensor_scalar_add(q_all[:, off:off + w],
                                                ps[:, :w], bq_t[:])
                    ps = p1.tile([128, 512], F32, name="kproj", tag="p1")
                    for c2 in range(2):
                        nc.tensor.matmul(ps[:, :w], wkt[c2][:],
                                         xf[c2][:, off:off + w],
                                         start=(c2 == 0), stop=(c2 == 1))
                    nc.vector.tensor_scalar_add(k_all[:, off:off + w],
                                                ps[:, :w], bk_t[:])
                for nt in range(NMT):
                    ps = p1.tile([128, 512], F32, name="vproj", tag="p1")
                    for c2 in range(2):
                        nc.tensor.matmul(ps[:],
                                         xf[c2][:, 128 * nt:128 * (nt + 1)],
                                         wvt[c2][:],
                                         start=(c2 == 0), stop=(c2 == 1))
                    nc.vector.tensor_copy(vt_all[:, 512 * nt:512 * (nt + 1)],
                                          ps[:])

            # ---------- phase 2: attention + output projection ----------
            # n-chunks of width 512 (last 256); heads processed two at a
            # time (half-chunks). Per m-tile step, the two heads' S^T
            # matmuls land in the two banks of one [128,1024] psum tile
            # (alternating between two such tiles) and a single exp call
            # evicts both to a bf16 pexp tile. PV runs one step behind the
            # QKs so the PE never waits on ACT; consecutive pexp steps are
            # pair-summed on DVE (bf16 2x) into resident t-tiles, which a
            # 2-more-level DVE tree folds to 3 tiles so the denominator
            # needs only 3 ones-matmuls. The finish phase (reciprocal,
            # broadcast, normalize, fp16 relu+bias on DVE) runs chunk-wide
            # in two dedicated psum banks, overlapping the next
            # half-chunk's loop. The output projection needs all four
            # heads, so it runs after the second half-chunk.
            NP = NMT // 2
            JCHUNKS = [(0, 512), (512, 512), (1024, 512), (1536, 512),
                       (2048, 256)]
            with tc.tile_pool(name="stp", bufs=1, space="PSUM") as stp, \
                 tc.tile_pool(name="xxp", bufs=1, space="PSUM") as xxp, \
                 tc.tile_pool(name="finp", bufs=1, space="PSUM") as finp:
                for joff, JW in JCHUNKS:
                    SW = 512 + JW      # combined two-head column width
                    r_ts = [None] * NHG
                    for ha, hb in ((0, 1), (2, 3)):
                        pi = ha // 2   # pair index 0/1
                        xx = {}
                        for h in (ha, hb):
                            xx[h] = xxp.tile([128, 512], F32, name=f"xx{h % 2}",
                                             tag=f"xx{h % 2}")[:, 0:JW]
                        pexp = [None] * NMT
                        tsum = [None] * NP
                        usum = [None] * 4
                        wsum = [None] * 2

                        def emit_qk_exp(mt):
                            # the two heads' matmuls run concurrently
                            # (different row groups), so they must land in
                            # different psum banks: halves at 0 and 512.
                            st = stp.tile([128, 1024], F32, name="st",
                                          tag=f"st{mt % 2}")
                            moff = 128 * mt
                            for i, h in enumerate((ha, hb)):
                                nc.tensor.matmul(
                                    st[:, 512 * i:512 * i + JW],
                                    k_all[32 * h:32 * (h + 1), moff:moff + 128],
                                    q_all[32 * h:32 * (h + 1), joff:joff + JW],
                                    start=True, stop=True,
                                    tile_position=(32 * h, 0))
                            pe = pexpp.tile([128, 1024], BF16, name="pe",
                                            tag="pe", bufs=3)
                            if JW == 512:
                                nc.scalar.activation(pe[:, 0:1024],
                                                     st[:, 0:1024], AF.Exp)
                            else:
                                st_v = st.rearrange("p (a b) -> p a b",
                                                    b=512)[:, :, 0:JW]
                                pe_v = pe.rearrange("p (a b) -> p a b",
                                                    b=512)[:, :, 0:JW]
                                nc.scalar.activation(pe_v, st_v, AF.Exp)
                            pexp[mt] = pe

                        def emit_pv(mt):
                            pe = pexp[mt]
                            for i, h in enumerate((ha, hb)):
                                nc.tensor.matmul(
                                    xx[h],
                                    vt_all[:, 512 * mt + 128 * h:
                                           512 * mt + 128 * (h + 1)],
                                    pe[:, 512 * i:512 * i + JW],
                                    start=(mt == 0), stop=(mt == NMT - 1))

                        def emit_tree(k):
                            # level-2/3 folds as soon as inputs are ready
                            if k % 2 == 1 and k < 8:
                                u = k // 2
                                t = workp.tile([128, 1024], BF16,
                                               name=f"u{u}", tag=f"u{u}",
                                               bufs=1)
                                nc.vector.tensor_tensor(
                                    t[:, 0:SW], tsum[k - 1][:, 0:SW],
                                    tsum[k][:, 0:SW], OP.add)
                                usum[u] = t
                                if u % 2 == 1:
                                    w_ = u // 2
                                    t2 = workp.tile([128, 1024], BF16,
                                                    name=f"w{w_}",
                                                    tag=f"w{w_}", bufs=1)
                                    nc.vector.tensor_tensor(
                                        t2[:, 0:SW], usum[u - 1][:, 0:SW],
                                        usum[u][:, 0:SW], OP.add)
                                    wsum[w_] = t2

                        for mt in range(NMT):
                            emit_qk_exp(mt)
                            if mt > 0:
                                emit_pv(mt - 1)
                            if mt % 2 == 1:
                                k = mt // 2
                                t = workp.tile([128, 1024], BF16,
                                               name=f"t{k}", tag=f"t{k}",
                                               bufs=1)
                                nc.vector.tensor_tensor(
                                    t[:, 0:SW],
                                    pexp[mt - 1][:, 0:SW],
                                    pexp[mt][:, 0:SW], OP.add)
                                tsum[k] = t
                                emit_tree(k)
                        emit_pv(NMT - 1)

                        # finish: denominators, normalize, bias+relu.
                        # Fold t8 into w1 so 2 ones-matmuls per head
                        # accumulate the column sums; reciprocal runs on the
                        # [1,JW] sums row and GPSIMD broadcasts it across
                        # partitions (no broadcast matmul, no s_row copy).
                        w1f = workp.tile([128, 1024], BF16, name="w1f",
                                         tag="w1f", bufs=1)
                        nc.vector.tensor_tensor(w1f[:, 0:SW],
                                                wsum[1][:, 0:SW],
                                                tsum[8][:, 0:SW], OP.add)
                        dens = [wsum[0], w1f]
                        for i, h in enumerate((ha, hb)):
                            fslot = f"f{i}"
                            sums_h = finp.tile([1, 512], F32, name="sums_h",
                                               tag=fslot)
                            for k, dt_ in enumerate(dens):
                                nc.tensor.matmul(
                                    sums_h[:, 0:JW], ones_bf[:],
                                    dt_[:, 512 * i:512 * i + JW],
                                    start=(k == 0),
                                    stop=(k == len(dens) - 1))
                            s_inv = workp.tile([1, 512], F32, name="s_inv",
                                               tag="s_inv")
                            nc.vector.reciprocal_approx_fast(s_inv[:, 0:JW],
                                                             sums_h[:, 0:JW])
                            inv_bc = workp.tile([128, 512], F32,
                                                name="inv_bc", tag="inv_bc")
                            nc.gpsimd.partition_broadcast(inv_bc[:, 0:JW],
                                                          s_inv[:, 0:JW])
                            t_h = workp.tile([128, 512], F32, name="t_h",
                                             tag="t_h")
                            nc.vector.tensor_tensor(t_h[:, 0:JW], xx[h],
                                                    inv_bc[:, 0:JW], OP.mult)
                            r_h = rp.tile([128, 512], FP16, name=f"r{h}",
                                          tag=f"r{h}")
                            nc.vector.tensor_scalar(
                                r_h[:, 0:JW], t_h[:, 0:JW], bv_t[h][:], 0.0,
                                OP.add, OP.max)
                            r_ts[h] = r_h

                    # output projection over all four heads
                    for ct in range(2):
                        op_ps = finp.tile([128, 512], F32, name="op_ps",
                                          tag=f"f{ct}")
                        for h in range(NHG):
                            nc.tensor.matmul(
                                op_ps[:, 0:JW],
                                wpt[h][:, 128 * ct:128 * (ct + 1)],
                                r_ts[h][:, 0:JW],
                                start=(h == 0), stop=(h == NHG - 1))
                        o_sb = workp.tile([128, 512], F32, name="o_sb",
                                          tag="o_sb")
                        nc.vector.tensor_copy(o_sb[:, 0:JW], op_ps[:, 0:JW])
                        nc.sync.dma_start(
                            out_d.ap()[128 * ct:128 * (ct + 1), joff:joff + JW],
                            o_sb[:, 0:JW])
    nc.compile()
    return nc


def _prep_inputs(x, wq, gq, bq, wk, gk, bk, wv, gv, bv, wp, gp, bp):
    """Fold BN scales into weights; build the 8 per-core input maps."""
    rs = np.float32(1.0 / np.sqrt(np.float32(1.0) + np.float32(EPS)))
    sq = (gq * rs).astype(np.float32)
    sk = (gk * rs).astype(np.float32)
    sv = (gv * rs).astype(np.float32)
    sp = (gp * rs).astype(np.float32)
    wq_f = (wq * sq[:, None]).astype(np.float16)
    wk_f = (wk * sk[:, None]).astype(np.float16)
    wv_f = (wv * sv[:, None]).astype(np.float16)
    wp_f = (wp * sp[:, None]).astype(np.float16)

    xf = np.ascontiguousarray(x.reshape(B, C, N).astype(np.float16))
    consts = np.zeros((128, 128 + J), dtype=np.float32)
    consts[:, 0:128] = 1.0
    import ml_dtypes
    ones_bf = np.ones((128, 1), dtype=ml_dtypes.bfloat16)
    in_maps = []
    for core in range(8):
        b, g = core // 2, core % 2
        qs = slice(128 * g, 128 * (g + 1))       # q/k rows for this head group
        vs = slice(512 * g, 512 * (g + 1))       # v rows / p cols for this group
        in_maps.append({
            "x_in": xf[b],
            "wqt": np.ascontiguousarray(wq_f[qs, :].T),
            "wkt": np.ascontiguousarray(wk_f[qs, :].T),
            "wvt": np.ascontiguousarray(wv_f[vs, :].T),
            "wpt": np.ascontiguousarray(wp_f[:, vs].T),
            "bq": np.ascontiguousarray(bq[qs].astype(np.float32)[:, None]),
            "bk": np.ascontiguousarray(bk[qs].astype(np.float32)[:, None]),
            "bv": np.ascontiguousarray(bv[vs].astype(np.float32)[:, None]),
            "consts": consts,
            "ones_bf": ones_bf,
        })
    return in_maps


def kernel(**inputs):
    if "nc" not in _CACHE:
        _CACHE["nc"] = _build_program()
    nc = _CACHE["nc"]

    in_maps = _prep_inputs(**{k: np.asarray(v) for k, v in inputs.items()})
    res = run_bass_kernel_spmd(nc, in_maps, list(range(8)))
    _CACHE["last_results"] = res

    bp = np.asarray(inputs["bp"]).astype(np.float32)
    out = np.empty((B, C, H, W), dtype=np.float32)
    for b in range(B):
        acc = res.results[2 * b]["outp"] + res.results[2 * b + 1]["outp"]
        acc = acc + bp[:, None]
        out[b] = acc.reshape(C, H, W)
    return out


# revision 16
# speedup vs baseline: 1.2472x; 1.0041x over previous
"""Trainium2 Bass kernel for the BN-attention module (nn_Attention).

Full inputs -> full output. Sharding: 8 cores = (batch b in 0..3) x
(head-group g in 0..1, 4 heads each). Each core computes its batch's
4-head attention and a partial output projection; the host sums the two
head-group partials per batch and adds the projection BN bias.

Numerics: BN scales are folded into the weights on the host. The Q/K
path (x, wq, wk, q, k) runs in fp16 (11-bit mantissa; logit error
~4e-3 absolute). The V/P paths run 16-bit as well: vT and exp(S^T) are
bf16 (exp needs bf16 range, up to e^~25); relu output and wp are fp16.
Softmax skips the max-subtraction (logits are O(25), safe in fp32
accumulation); both attn@V and the softmax denominators consume the
same bf16 exp values so their rounding largely cancels in the ratio.

Layout: attention is computed transposed, S^T = K^T Q with keys (m) on
partitions, so attn@V needs no transposes at all: V is produced
directly as vT[n,d] by the projection. Denominators are column sums:
exp tiles are pair-summed on the VectorE (bf16 2x) into a 3-level
tree (9 -> 4+t8 -> 2+t8), then 3 ones-matmuls accumulate the column
sums on the TensorE. The finish phase (reciprocal, broadcast,
normalize, bias+relu) runs chunk-wide: one reciprocal per [128, 1024]
chunk, relu+bias as a dual-op tensor_scalar on the VectorE (fp16 out),
keeping the ScalarE exp-only.
"""

import numpy as np

import concourse.bacc as bacc
import concourse.mybir as mybir
import concourse.tile as tile
from concourse.bass_utils import run_bass_kernel_spmd

# Problem dims (hardcoded per the spec)
B, C, H, W = 4, 256, 48, 48
N = H * W            # 2304
KD, NH, AR = 32, 8, 4
D = AR * KD          # 128 value dims per head
NHKD = NH * KD       # 256
DH = NH * D          # 1024
EPS = 1e-5

NHG = 4              # heads per core
J = 256              # (legacy) consts tile width
MT = 128             # m-tile (key tile)
NMT = N // MT        # 18

F32 = mybir.dt.float32
F32R = mybir.dt.float32r
BF16 = mybir.dt.bfloat16
FP16 = mybir.dt.float16
AF = mybir.ActivationFunctionType
OP = mybir.AluOpType

# chunks of 2304 by <=512 for the projection matmuls
CHUNKS_512 = [(off, min(512, N - off)) for off in range(0, N, 512)]

_CACHE = {}


def _build_program():
    nc = bacc.Bacc("TRN2", target_bir_lowering=False, debug=False)

    x_in = nc.dram_tensor("x_in", [C, N], FP16, kind="ExternalInput")
    wqt_d = nc.dram_tensor("wqt", [C, 128], FP16, kind="ExternalInput")
    wkt_d = nc.dram_tensor("wkt", [C, 128], FP16, kind="ExternalInput")
    wvt_d = nc.dram_tensor("wvt", [C, 512], FP16, kind="ExternalInput")
    wpt_d = nc.dram_tensor("wpt", [512, C], FP16, kind="ExternalInput")
    bq_d = nc.dram_tensor("bq", [128, 1], F32, kind="ExternalInput")
    bk_d = nc.dram_tensor("bk", [128, 1], F32, kind="ExternalInput")
    bv_d = nc.dram_tensor("bv", [512, 1], F32, kind="ExternalInput")
    consts_d = nc.dram_tensor("consts", [128, 128 + J], F32R, kind="ExternalInput")
    ones_bf_d = nc.dram_tensor("ones_bf", [128, 1], BF16, kind="ExternalInput")
    out_d = nc.dram_tensor("outp", [C, N], F32, kind="ExternalOutput")

    with tile.TileContext(nc) as tc:
        with nc.allow_low_precision(reason="16-bit matmul rounding is intentional"), \
             tc.tile_pool(name="const", bufs=1) as constp, \
             tc.tile_pool(name="qk", bufs=1) as qkp, \
             tc.tile_pool(name="vt", bufs=1) as vtp, \
             tc.tile_pool(name="pexp", bufs=1) as pexpp, \
             tc.tile_pool(name="rp", bufs=1) as rp, \
             tc.tile_pool(name="work", bufs=2) as workp:

            # ---------- constants / inputs ----------
            # DMA order matters: the q/k weights + biases + x chunk 0 go
            # first so phase 1 starts as early as possible.
            wqt, wkt, wvt = [], [], []
            for c2 in range(2):
                sl = slice(128 * c2, 128 * (c2 + 1))
                t = constp.tile([128, 128], FP16, name=f"wqt{c2}", tag=f"wqt{c2}")
                nc.sync.dma_start(t[:], wqt_d.ap()[sl, :])
                wqt.append(t)
                t = constp.tile([128, 128], FP16, name=f"wkt{c2}", tag=f"wkt{c2}")
                nc.sync.dma_start(t[:], wkt_d.ap()[sl, :])
                wkt.append(t)
            bq_t = constp.tile([128, 1], F32, name="bq_t", tag="bq_t")
            nc.scalar.dma_start(bq_t[:], bq_d.ap())
            bk_t = constp.tile([128, 1], F32, name="bk_t", tag="bk_t")
            nc.scalar.dma_start(bk_t[:], bk_d.ap())
            xf = [constp.tile([128, N], FP16, name=f"xf{c2}", tag=f"xf{c2}")
                  for c2 in range(2)]
            for off, w in CHUNKS_512:
                nc.sync.dma_start(xf[0][:, off:off + w],
                                  x_in.ap()[0:128, off:off + w])
                nc.scalar.dma_start(xf[1][:, off:off + w],
                                    x_in.ap()[128:256, off:off + w])
                if off == 0:
                    for c2 in range(2):
                        t = constp.tile([128, 512], FP16, name=f"wvt{c2}",
                                        tag=f"wvt{c2}")
                        eng = nc.sync if c2 == 0 else nc.scalar
                        eng.dma_start(t[:], wvt_d.ap()[
                            128 * c2:128 * (c2 + 1), :])
                        wvt.append(t)
            wpt = []
            for h in range(NHG):
                t = constp.tile([128, C], FP16, name=f"wpt{h}", tag=f"wpt{h}")
                eng = nc.sync if h % 2 == 0 else nc.scalar
                eng.dma_start(t[:], wpt_d.ap()[128 * h:128 * (h + 1), :])
                wpt.append(t)
            bv_t = []
            for h in range(NHG):
                t = constp.tile([128, 1], F32, name=f"bv{h}", tag=f"bv{h}")
                nc.sync.dma_start(t[:], bv_d.ap()[128 * h:128 * (h + 1), :])
                bv_t.append(t)
            # constants via DMA (memset can't produce float32r)
            consts_t = constp.tile([128, 128 + J], F32R, name="consts_t",
                                   tag="consts_t")
            nc.sync.dma_start(consts_t[:], consts_d.ap())
            ones_all = consts_t[:, 0:128]     # [128,128] of 1.0
            ones_bf = constp.tile([128, 1], BF16, name="ones_bf", tag="ones_bf")
            nc.sync.dma_start(ones_bf[:], ones_bf_d.ap())

            q_all = qkp.tile([128, N], FP16, name="q_all", tag="q_all")
            k_all = qkp.tile([128, N], FP16, name="k_all", tag="k_all")
            vt_all = vtp.tile([128, NMT * 512], BF16, name="vt_all", tag="vt_all")

            # ---------- phase 1: q/k/v projections ----------
            with tc.tile_pool(name="p1", bufs=4, space="PSUM") as p1:
                for off, w in CHUNKS_512:
                    ps = p1.tile([128, 512], F32, name="qproj", tag="p1")
                    for c2 in range(2):
                        nc.tensor.matmul(ps[:, :w], wqt[c2][:],
                                         xf[c2][:, off:off + w],
                                         start=(c2 == 0), stop=(c2 == 1))
                    nc.vector.tensor_scalar_add(q_all[:, off:off + w],
                                                ps[:, :w], bq_t[:])
                    ps = p1.tile([128, 512], F32, name="kproj", tag="p1")
                    for c2 in range(2):
                        nc.tensor.matmul(ps[:, :w], wkt[c2][:],
                                         xf[c2][:, off:off + w],
                                         start=(c2 == 0), stop=(c2 == 1))
                    nc.vector.tensor_scalar_add(k_all[:, off:off + w],
                                                ps[:, :w], bk_t[:])
                for nt in range(NMT):
                    ps = p1.tile([128, 512], F32, name="vproj", tag="p1")
                    for c2 in range(2):
                        nc.tensor.matmul(ps[:],
                                         xf[c2][:, 128 * nt:128 * (nt + 1)],
                                         wvt[c2][:],
                                         start=(c2 == 0), stop=(c2 == 1))
                    # ScalarE is idle in phase 1; use it for the v evictions
                    nc.scalar.copy(vt_all[:, 512 * nt:512 * (nt + 1)], ps[:])

            # ---------- phase 2: attention + output projection ----------
            # n-chunks of width 512 (last 256); heads processed two at a
            # time (half-chunks). Per m-tile step, the two heads' S^T
            # matmuls land in the two banks of one [128,1024] psum tile
            # (alternating between two such tiles) and a single exp call
            # evicts both to a bf16 pexp tile. PV runs one step behind the
            # QKs so the PE never waits on ACT; consecutive pexp steps are
            # pair-summed on DVE (bf16 2x) into resident t-tiles, which a
            # 2-more-level DVE tree folds to 3 tiles so the denominator
            # needs only 3 ones-matmuls. The finish phase (reciprocal,
            # broadcast, normalize, fp16 relu+bias on DVE) runs chunk-wide
            # in two dedicated psum banks, overlapping the next
            # half-chunk's loop. The output projection needs all four
            # heads, so it runs after the second half-chunk.
            NP = NMT // 2
            G2 = ((0, 1), (2, 3))
            JCHUNKS = [(0, 512, G2), (512, 512, G2), (1024, 512, G2),
                       (1536, 512, G2), (2048, 256, G2)]
            with tc.tile_pool(name="stp", bufs=1, space="PSUM") as stp, \
                 tc.tile_pool(name="xxp", bufs=1, space="PSUM") as xxp, \
                 tc.tile_pool(name="finp", bufs=1, space="PSUM") as finp:
                def emit_proj(joff_p, JW_p, r_p):
                    # output projection over all four heads of a finished
                    # chunk (deferred into the next chunk's loop so the PE
                    # never stalls on the finish chain at chunk boundaries)
                    for ct in range(2):
                        op_ps = finp.tile([128, 512], F32, name="op_ps",
                                          tag=f"f{ct}")
                        for h in range(NHG):
                            nc.tensor.matmul(
                                op_ps[:, 0:JW_p],
                                wpt[h][:, 128 * ct:128 * (ct + 1)],
                                r_p[h][:, 0:JW_p],
                                start=(h == 0), stop=(h == NHG - 1))
                        o_sb = workp.tile([128, 512], F32, name="o_sb",
                                          tag="o_sb")
                        nc.vector.tensor_copy(o_sb[:, 0:JW_p],
                                              op_ps[:, 0:JW_p])
                        nc.sync.dma_start(
                            out_d.ap()[128 * ct:128 * (ct + 1),
                                       joff_p:joff_p + JW_p],
                            o_sb[:, 0:JW_p])

                pending = None     # (joff, JW, r_ts) of the previous chunk
                for joff, JW, groups in JCHUNKS:
                    r_ts = [None] * NHG
                    for gi, grp in enumerate(groups):
                        G = len(grp)           # heads in this group
                        xxt = [xxp.tile([128, 512], F32, name=f"xx{j}",
                                        tag=f"xx{j}") for j in range(2)]
                        xx = {}
                        for i, h in enumerate(grp):
                            xx[h] = xxt[i][:, 0:JW]
                        pexp = [None] * NMT
                        tsum = [None] * NP
                        usum = [None] * 4
                        wsum = [None] * 2

                        def emit_qk_exp(mt):
                            # the G heads' matmuls run concurrently
                            # (different PE row groups); each head's slice
                            # stays within a psum bank.
                            st = stp.tile([128, 1024], F32, name="st",
                                          tag=f"st{mt % 2}")
                            moff = 128 * mt
                            for i, h in enumerate(grp):
                                nc.tensor.matmul(
                                    st[:, 512 * i:512 * i + JW],
                                    k_all[32 * h:32 * (h + 1), moff:moff + 128],
                                    q_all[32 * h:32 * (h + 1), joff:joff + JW],
                                    start=True, stop=True,
                                    tile_position=(32 * h, 0))
                            pe = pexpp.tile([128, 1024], BF16, name="pe",
                                            tag="pe", bufs=3)
                            if JW == 512:
                                nc.scalar.activation(pe[:, 0:1024],
                                                     st[:, 0:1024], AF.Exp)
                            else:
                                st_v = st.rearrange("p (a b) -> p a b",
                                                    b=512)[:, :, 0:JW]
                                pe_v = pe.rearrange("p (a b) -> p a b",
                                                    b=512)[:, :, 0:JW]
                                nc.scalar.activation(pe_v, st_v, AF.Exp)
                            pexp[mt] = pe

                        def emit_pv(mt):
                            pe = pexp[mt]
                            for i, h in enumerate(grp):
                                nc.tensor.matmul(
                                    xx[h],
                                    vt_all[:, 512 * mt + 128 * h:
                                           512 * mt + 128 * (h + 1)],
                                    pe[:, 512 * i:512 * i + JW],
                                    start=(mt == 0), stop=(mt == NMT - 1))

                        def emit_tree(k):
                            # level-2/3 folds as soon as inputs are ready
                            if k % 2 == 1 and k < 8:
                                u = k // 2
                                t = workp.tile([128, 1024], BF16,
                                               name=f"u{u}", tag=f"u{u}",
                                               bufs=1)
                                nc.vector.tensor_tensor(
                                    t[:], tsum[k - 1][:], tsum[k][:], OP.add)
                                usum[u] = t
                                if u % 2 == 1:
                                    w_ = u // 2
                                    t2 = workp.tile([128, 1024], BF16,
                                                    name=f"w{w_}",
                                                    tag=f"w{w_}", bufs=1)
                                    nc.vector.tensor_tensor(
                                        t2[:], usum[u - 1][:], usum[u][:],
                                        OP.add)
                                    wsum[w_] = t2

                        for mt in range(NMT):
                            emit_qk_exp(mt)
                            if mt > 0:
                                emit_pv(mt - 1)
                            if mt == 3 and gi == 0 and pending is not None:
                                emit_proj(*pending)
                                pending = None
                            if mt % 2 == 1:
                                k = mt // 2
                                t = workp.tile([128, 1024], BF16,
                                               name=f"t{k}", tag=f"t{k}",
                                               bufs=1)
                                nc.vector.tensor_tensor(
                                    t[:], pexp[mt - 1][:], pexp[mt][:],
                                    OP.add)
                                tsum[k] = t
                                emit_tree(k)
                        emit_pv(NMT - 1)

                        # finish: denominators, normalize, bias+relu.
                        # Fold t8 into w1 so 2 ones-matmuls per head
                        # accumulate the column sums; reciprocal runs on the
                        # [1,JW] sums row and GPSIMD broadcasts it across
                        # partitions (no broadcast matmul, no s_row copy).
                        w1f = workp.tile([128, 1024], BF16, name="w1f",
                                         tag="w1f", bufs=1)
                        nc.vector.tensor_tensor(w1f[:], wsum[1][:],
                                                tsum[8][:], OP.add)
                        dens = [wsum[0], w1f]
                        for i, h in enumerate(grp):
                            fslot = f"f{i % 2}"
                            sums_h = finp.tile([1, 512], F32, name="sums_h",
                                               tag=fslot)
                            for k, dt_ in enumerate(dens):
                                nc.tensor.matmul(
                                    sums_h[:, 0:JW], ones_bf[:],
                                    dt_[:, 512 * i:512 * i + JW],
                                    start=(k == 0),
                                    stop=(k == len(dens) - 1))
                            s_inv = workp.tile([1, 512], F32, name="s_inv",
                                               tag="s_inv")
                            nc.vector.reciprocal_approx_fast(s_inv[:, 0:JW],
                                                             sums_h[:, 0:JW])
                            inv_bc = workp.tile([128, 512], F32,
                                                name="inv_bc", tag="inv_bc")
                            nc.gpsimd.partition_broadcast(inv_bc[:, 0:JW],
                                                          s_inv[:, 0:JW])
                            t_h = workp.tile([128, 512], F32, name="t_h",
                                             tag="t_h")
                            nc.vector.tensor_tensor(t_h[:, 0:JW], xx[h],
                                                    inv_bc[:, 0:JW], OP.mult)
                            r_h = rp.tile([128, 512], FP16, name=f"r{h}",
                                          tag=f"r{h}")
                            nc.vector.tensor_scalar(
                                r_h[:, 0:JW], t_h[:, 0:JW], bv_t[h][:], 0.0,
                                OP.add, OP.max)
                            r_ts[h] = r_h

                    pending = (joff, JW, list(r_ts))
                emit_proj(*pending)
    nc.compile()
    return nc


def _prep_inputs(x, wq, gq, bq, wk, gk, bk, wv, gv, bv, wp, gp, bp):
    """Fold BN scales into weights; build the 8 per-core input maps."""
    rs = np.float32(1.0 / np.sqrt(np.float32(1.0) + np.float32(EPS)))
    sq = (gq * rs).astype(np.float32)
    sk = (gk * rs).astype(np.float32)
    sv = (gv * rs).astype(np.float32)
    sp = (gp * rs).astype(np.float32)
    wq_f = (wq * sq[:, None]).astype(np.float16)
    wk_f = (wk * sk[:, None]).astype(np.float16)
    wv_f = (wv * sv[:, None]).astype(np.float16)
    wp_f = (wp * sp[:, None]).astype(np.float16)

    xf = np.ascontiguousarray(x.reshape(B, C, N).astype(np.float16))
    consts = np.zeros((128, 128 + J), dtype=np.float32)
    consts[:, 0:128] = 1.0
    import ml_dtypes
    ones_bf = np.ones((128, 1), dtype=ml_dtypes.bfloat16)
    in_maps = []
    for core in range(8):
        b, g = core // 2, core % 2
        qs = slice(128 * g, 128 * (g + 1))       # q/k rows for this head group
        vs = slice(512 * g, 512 * (g + 1))       # v rows / p cols for this group
        in_maps.append({
            "x_in": xf[b],
            "wqt": np.ascontiguousarray(wq_f[qs, :].T),
            "wkt": np.ascontiguousarray(wk_f[qs, :].T),
            "wvt": np.ascontiguousarray(wv_f[vs, :].T),
            "wpt": np.ascontiguousarray(wp_f[:, vs].T),
            "bq": np.ascontiguousarray(bq[qs].astype(np.float32)[:, None]),
            "bk": np.ascontiguousarray(bk[qs].astype(np.float32)[:, None]),
            "bv": np.ascontiguousarray(bv[vs].astype(np.float32)[:, None]),
            "consts": consts,
            "ones_bf": ones_bf,
        })
    return in_maps


def kernel(**inputs):
    if "nc" not in _CACHE:
        _CACHE["nc"] = _build_program()
    nc = _CACHE["nc"]

    in_maps = _prep_inputs(**{k: np.asarray(v) for k, v in inputs.items()})
    res = run_bass_kernel_spmd(nc, in_maps, list(range(8)))
    _CACHE["last_results"] = res

    bp = np.asarray(inputs["bp"]).astype(np.float32)
    out = np.empty((B, C, H, W), dtype=np.float32)
    for b in range(B):
        acc = res.results[2 * b]["outp"] + res.results[2 * b + 1]["outp"]
        acc = acc + bp[:, None]
        out[b] = acc.reshape(C, H, W)
    return out


# revision 21
# speedup vs baseline: 1.3217x; 1.0598x over previous
"""Trainium2 Bass kernel for the BN-attention module (nn_Attention).

Full inputs -> full output. Sharding: 8 cores = (batch b in 0..3) x
(head-group g in 0..1, 4 heads each). Each core computes its batch's
4-head attention and a partial output projection; the host sums the two
head-group partials per batch and adds the projection BN bias.

Numerics: BN scales are folded into the weights on the host. The Q/K
path (x, wq, wk, q, k) runs in fp16 (11-bit mantissa; logit error
~4e-3 absolute). The V/P paths run 16-bit as well: vT and exp(S^T) are
bf16 (exp needs bf16 range, up to e^~25); relu output and wp are fp16.
Softmax skips the max-subtraction (logits are O(25), safe in fp32
accumulation); both attn@V and the softmax denominators consume the
same bf16 exp values so their rounding largely cancels in the ratio.

Layout: attention is computed transposed, S^T = K^T Q with keys (m) on
partitions, so attn@V needs no transposes at all: V is produced
directly as vT[n,d] by the projection. Denominators are column sums:
exp tiles are pair-summed on the VectorE (bf16 2x) into a 3-level
tree (9 -> 4+t8 -> 2+t8), then 3 ones-matmuls accumulate the column
sums on the TensorE. The finish phase (reciprocal, broadcast,
normalize, bias+relu) runs chunk-wide: one reciprocal per [128, 1024]
chunk, relu+bias as a dual-op tensor_scalar on the VectorE (fp16 out),
keeping the ScalarE exp-only.
"""

import numpy as np

import concourse.bacc as bacc
import concourse.mybir as mybir
import concourse.tile as tile
from concourse.bass_utils import run_bass_kernel_spmd

# Problem dims (hardcoded per the spec)
B, C, H, W = 4, 256, 48, 48
N = H * W            # 2304
KD, NH, AR = 32, 8, 4
D = AR * KD          # 128 value dims per head
NHKD = NH * KD       # 256
DH = NH * D          # 1024
EPS = 1e-5

NHG = 4              # heads per core
J = 256              # (legacy) consts tile width
MT = 128             # m-tile (key tile)
NMT = N // MT        # 18

F32 = mybir.dt.float32
F32R = mybir.dt.float32r
BF16 = mybir.dt.bfloat16
FP16 = mybir.dt.float16
AF = mybir.ActivationFunctionType
OP = mybir.AluOpType

# chunks of 2304 by <=512 for the projection matmuls
CHUNKS_512 = [(off, min(512, N - off)) for off in range(0, N, 512)]

_CACHE = {}


def _build_program():
    nc = bacc.Bacc("TRN2", target_bir_lowering=False, debug=False)

    x_in = nc.dram_tensor("x_in", [C, N], FP16, kind="ExternalInput")
    wqt_d = nc.dram_tensor("wqt", [C, 128], FP16, kind="ExternalInput")
    wkt_d = nc.dram_tensor("wkt", [C, 128], FP16, kind="ExternalInput")
    wvt_d = nc.dram_tensor("wvt", [C, 512], FP16, kind="ExternalInput")
    wpt_d = nc.dram_tensor("wpt", [512, C], FP16, kind="ExternalInput")
    bq_d = nc.dram_tensor("bq", [128, 1], F32, kind="ExternalInput")
    bk_d = nc.dram_tensor("bk", [128, 1], F32, kind="ExternalInput")
    bv_d = nc.dram_tensor("bv", [512, 1], F32, kind="ExternalInput")
    consts_d = nc.dram_tensor("consts", [128, 128 + J], F32R, kind="ExternalInput")
    ones_bf_d = nc.dram_tensor("ones_bf", [128, 1], BF16, kind="ExternalInput")
    out_d = nc.dram_tensor("outp", [C, N], F32, kind="ExternalOutput")

    with tile.TileContext(nc) as tc:
        with nc.allow_low_precision(reason="16-bit matmul rounding is intentional"), \
             tc.tile_pool(name="const", bufs=1) as constp, \
             tc.tile_pool(name="qk", bufs=1) as qkp, \
             tc.tile_pool(name="vt", bufs=1) as vtp, \
             tc.tile_pool(name="pexp", bufs=1) as pexpp, \
             tc.tile_pool(name="rp", bufs=1) as rp, \
             tc.tile_pool(name="work", bufs=2) as workp:

            # ---------- constants / inputs ----------
            # DMA order matters: the q/k weights + biases + x chunk 0 go
            # first so phase 1 starts as early as possible.
            wqt, wkt, wvt = [], [], []
            for c2 in range(2):
                sl = slice(128 * c2, 128 * (c2 + 1))
                t = constp.tile([128, 128], FP16, name=f"wqt{c2}", tag=f"wqt{c2}")
                nc.sync.dma_start(t[:], wqt_d.ap()[sl, :])
                wqt.append(t)
                t = constp.tile([128, 128], FP16, name=f"wkt{c2}", tag=f"wkt{c2}")
                nc.sync.dma_start(t[:], wkt_d.ap()[sl, :])
                wkt.append(t)
            bq_t = constp.tile([128, 1], F32, name="bq_t", tag="bq_t")
            nc.scalar.dma_start(bq_t[:], bq_d.ap())
            bk_t = constp.tile([128, 1], F32, name="bk_t", tag="bk_t")
            nc.scalar.dma_start(bk_t[:], bk_d.ap())
            xf = [constp.tile([128, N], FP16, name=f"xf{c2}", tag=f"xf{c2}")
                  for c2 in range(2)]
            for off, w in CHUNKS_512:
                nc.sync.dma_start(xf[0][:, off:off + w],
                                  x_in.ap()[0:128, off:off + w])
                nc.scalar.dma_start(xf[1][:, off:off + w],
                                    x_in.ap()[128:256, off:off + w])
                if off == 0:
                    for c2 in range(2):
                        t = constp.tile([128, 512], FP16, name=f"wvt{c2}",
                                        tag=f"wvt{c2}")
                        eng = nc.sync if c2 == 0 else nc.scalar
                        eng.dma_start(t[:], wvt_d.ap()[
                            128 * c2:128 * (c2 + 1), :])
                        wvt.append(t)
            wpt = []
            for h in range(NHG):
                t = constp.tile([128, C], FP16, name=f"wpt{h}", tag=f"wpt{h}")
                eng = nc.sync if h % 2 == 0 else nc.scalar
                eng.dma_start(t[:], wpt_d.ap()[128 * h:128 * (h + 1), :])
                wpt.append(t)
            bv_t = []
            for h in range(NHG):
                t = constp.tile([128, 1], F32, name=f"bv{h}", tag=f"bv{h}")
                nc.sync.dma_start(t[:], bv_d.ap()[128 * h:128 * (h + 1), :])
                bv_t.append(t)
            # constants via DMA (memset can't produce float32r)
            consts_t = constp.tile([128, 128 + J], F32R, name="consts_t",
                                   tag="consts_t")
            nc.sync.dma_start(consts_t[:], consts_d.ap())
            ones_all = consts_t[:, 0:128]     # [128,128] of 1.0
            ones_bf = constp.tile([128, 1], BF16, name="ones_bf", tag="ones_bf")
            nc.sync.dma_start(ones_bf[:], ones_bf_d.ap())

            q_all = qkp.tile([128, N], FP16, name="q_all", tag="q_all")
            k_all = qkp.tile([128, N], FP16, name="k_all", tag="k_all")
            vt_all = vtp.tile([128, NMT * 512], BF16, name="vt_all", tag="vt_all")

            # ---------- phase 1: q/k/v projections ----------
            with tc.tile_pool(name="p1", bufs=4, space="PSUM") as p1:
                for off, w in CHUNKS_512:
                    ps = p1.tile([128, 512], F32, name="qproj", tag="p1")
                    for c2 in range(2):
                        nc.tensor.matmul(ps[:, :w], wqt[c2][:],
                                         xf[c2][:, off:off + w],
                                         start=(c2 == 0), stop=(c2 == 1))
                    nc.vector.tensor_scalar_add(q_all[:, off:off + w],
                                                ps[:, :w], bq_t[:])
                    ps = p1.tile([128, 512], F32, name="kproj", tag="p1")
                    for c2 in range(2):
                        nc.tensor.matmul(ps[:, :w], wkt[c2][:],
                                         xf[c2][:, off:off + w],
                                         start=(c2 == 0), stop=(c2 == 1))
                    nc.vector.tensor_scalar_add(k_all[:, off:off + w],
                                                ps[:, :w], bk_t[:])
                # (v projections are interleaved into chunk 0's loop below
                # so the ScalarE starts exp work as early as possible)

            # ---------- phase 2: attention + output projection ----------
            # n-chunks of width 512 (last 256); heads processed two at a
            # time (half-chunks). Per m-tile step, the two heads' S^T
            # matmuls land in the two banks of one [128,1024] psum tile
            # (alternating between two such tiles) and a single exp call
            # evicts both to a bf16 pexp tile. PV runs one step behind the
            # QKs so the PE never waits on ACT; consecutive pexp steps are
            # pair-summed on DVE (bf16 2x) into resident t-tiles, which a
            # 2-more-level DVE tree folds to 3 tiles so the denominator
            # needs only 3 ones-matmuls. The finish phase (reciprocal,
            # broadcast, normalize, fp16 relu+bias on DVE) runs chunk-wide
            # in two dedicated psum banks, overlapping the next
            # half-chunk's loop. The output projection needs all four
            # heads, so it runs after the second half-chunk.
            NP = NMT // 2
            G2 = ((0, 1), (2, 3))
            JCHUNKS = [(0, 512, G2), (512, 512, G2), (1024, 512, G2),
                       (1536, 512, G2), (2048, 256, G2)]
            with tc.tile_pool(name="stp", bufs=1, space="PSUM") as stp, \
                 tc.tile_pool(name="xxp", bufs=1, space="PSUM") as xxp, \
                 tc.tile_pool(name="finp", bufs=1, space="PSUM") as finp:
                def emit_proj(joff_p, JW_p, r_p):
                    # output projection over all four heads of a finished
                    # chunk (deferred into the next chunk's loop so the PE
                    # never stalls on the finish chain at chunk boundaries)
                    for ct in range(2):
                        op_ps = finp.tile([128, 512], F32, name="op_ps",
                                          tag=f"f{ct}")
                        for h in range(NHG):
                            nc.tensor.matmul(
                                op_ps[:, 0:JW_p],
                                wpt[h][:, 128 * ct:128 * (ct + 1)],
                                r_p[h][:, 0:JW_p],
                                start=(h == 0), stop=(h == NHG - 1))
                        o_sb = workp.tile([128, 512], F32, name="o_sb",
                                          tag="o_sb")
                        nc.vector.tensor_copy(o_sb[:, 0:JW_p],
                                              op_ps[:, 0:JW_p])
                        nc.sync.dma_start(
                            out_d.ap()[128 * ct:128 * (ct + 1),
                                       joff_p:joff_p + JW_p],
                            o_sb[:, 0:JW_p])

                pending = None     # (joff, JW, r_ts) of the previous chunk
                for ci, (joff, JW, groups) in enumerate(JCHUNKS):
                    r_ts = [None] * NHG
                    for gi, grp in enumerate(groups):
                        G = len(grp)           # heads in this group
                        xxt = [xxp.tile([128, 512], F32, name=f"xx{j}",
                                        tag=f"xx{j}") for j in range(2)]
                        xx = {}
                        for i, h in enumerate(grp):
                            xx[h] = xxt[i][:, 0:JW]
                        pexp = [None] * NMT
                        tsum = [None] * NP
                        usum = [None] * 4
                        wsum = [None] * 2

                        def emit_qk_exp(mt):
                            # the G heads' matmuls run concurrently
                            # (different PE row groups); each head's slice
                            # stays within a psum bank.
                            st = stp.tile([128, 1024], F32, name="st",
                                          tag=f"st{mt % 2}")
                            moff = 128 * mt
                            for i, h in enumerate(grp):
                                nc.tensor.matmul(
                                    st[:, 512 * i:512 * i + JW],
                                    k_all[32 * h:32 * (h + 1), moff:moff + 128],
                                    q_all[32 * h:32 * (h + 1), joff:joff + JW],
                                    start=True, stop=True,
                                    tile_position=(32 * h, 0))
                            pe = pexpp.tile([128, 1024], BF16, name="pe",
                                            tag="pe", bufs=3)
                            if JW == 512:
                                nc.scalar.activation(pe[:, 0:1024],
                                                     st[:, 0:1024], AF.Exp)
                            else:
                                st_v = st.rearrange("p (a b) -> p a b",
                                                    b=512)[:, :, 0:JW]
                                pe_v = pe.rearrange("p (a b) -> p a b",
                                                    b=512)[:, :, 0:JW]
                                nc.scalar.activation(pe_v, st_v, AF.Exp)
                            pexp[mt] = pe

                        def emit_pv(mt):
                            pe = pexp[mt]
                            for i, h in enumerate(grp):
                                nc.tensor.matmul(
                                    xx[h],
                                    vt_all[:, 512 * mt + 128 * h:
                                           512 * mt + 128 * (h + 1)],
                                    pe[:, 512 * i:512 * i + JW],
                                    start=(mt == 0), stop=(mt == NMT - 1))

                        def emit_tree(k):
                            # level-2/3/4 folds as soon as inputs are ready;
                            # W = t0+..+t7 is complete by mt 15 so only t8
                            # (the last exp pair) is left for the finish.
                            if k % 2 == 1 and k < 8:
                                u = k // 2
                                t = workp.tile([128, 1024], BF16,
                                               name=f"u{u}", tag=f"u{u}",
                                               bufs=1)
                                nc.vector.tensor_tensor(
                                    t[:], tsum[k - 1][:], tsum[k][:], OP.add)
                                usum[u] = t
                                if u % 2 == 1:
                                    w_ = u // 2
                                    t2 = workp.tile([128, 1024], BF16,
                                                    name=f"w{w_}",
                                                    tag=f"w{w_}", bufs=1)
                                    nc.vector.tensor_tensor(
                                        t2[:], usum[u - 1][:], usum[u][:],
                                        OP.add)
                                    wsum[w_] = t2
                                    if w_ == 1:
                                        t3 = workp.tile([128, 1024], BF16,
                                                        name="wall",
                                                        tag="wall", bufs=1)
                                        nc.vector.tensor_tensor(
                                            t3[:], wsum[0][:], wsum[1][:],
                                            OP.add)
                                        wsum.append(t3)

                        sums_hs = [None, None]
                        for mt in range(NMT):
                            if ci == 0 and gi == 0:
                                # v projection for m-tile mt, one step ahead
                                # of its PV consumer; evicted on ScalarE
                                ps_v = finp.tile([128, 512], F32, name="vps",
                                                 tag=f"f{mt % 2}")
                                for c2 in range(2):
                                    nc.tensor.matmul(
                                        ps_v[:],
                                        xf[c2][:, 128 * mt:128 * (mt + 1)],
                                        wvt[c2][:],
                                        start=(c2 == 0), stop=(c2 == 1))
                                nc.scalar.copy(
                                    vt_all[:, 512 * mt:512 * (mt + 1)],
                                    ps_v[:])
                            emit_qk_exp(mt)
                            if mt > 0:
                                emit_pv(mt - 1)
                            if mt == 3 and gi == 0 and pending is not None:
                                emit_proj(*pending)
                                pending = None
                            if mt == NMT - 1:
                                # W covers t0..t7 (32 of 36 m-tiles); start
                                # the denominator accumulation while the
                                # last exp is still in flight.
                                for i in range(G):
                                    sums_hs[i] = finp.tile(
                                        [1, 512], F32, name="sums_h",
                                        tag=f"f{i % 2}")
                                    nc.tensor.matmul(
                                        sums_hs[i][:, 0:JW], ones_bf[:],
                                        wsum[2][:, 512 * i:512 * i + JW],
                                        start=True, stop=False)
                            if mt % 2 == 1:
                                k = mt // 2
                                t = workp.tile([128, 1024], BF16,
                                               name=f"t{k}", tag=f"t{k}",
                                               bufs=1)
                                nc.vector.tensor_tensor(
                                    t[:], pexp[mt - 1][:], pexp[mt][:],
                                    OP.add)
                                tsum[k] = t
                                emit_tree(k)
                        emit_pv(NMT - 1)

                        # finish: denominators, normalize, bias+relu.
                        # Only the t8 ones-matmul waits on the last exp;
                        # reciprocal runs on the [1,JW] sums row and GPSIMD
                        # broadcasts it across partitions.
                        for i, h in enumerate(grp):
                            sums_h = sums_hs[i]
                            nc.tensor.matmul(
                                sums_h[:, 0:JW], ones_bf[:],
                                tsum[8][:, 512 * i:512 * i + JW],
                                start=False, stop=True)
                            s_inv = workp.tile([1, 512], F32, name="s_inv",
                                               tag="s_inv")
                            nc.vector.reciprocal_approx_fast(s_inv[:, 0:JW],
                                                             sums_h[:, 0:JW])
                            inv_bc = workp.tile([128, 512], F32,
                                                name="inv_bc", tag="inv_bc")
                            nc.gpsimd.partition_broadcast(inv_bc[:, 0:JW],
                                                          s_inv[:, 0:JW])
                            t_h = workp.tile([128, 512], F32, name="t_h",
                                             tag="t_h")
                            nc.vector.tensor_tensor(t_h[:, 0:JW], xx[h],
                                                    inv_bc[:, 0:JW], OP.mult)
                            r_h = rp.tile([128, 512], FP16, name=f"r{h}",
                                          tag=f"r{h}")
                            nc.vector.tensor_scalar(
                                r_h[:, 0:JW], t_h[:, 0:JW], bv_t[h][:], 0.0,
                                OP.add, OP.max)
                            r_ts[h] = r_h

                    pending = (joff, JW, list(r_ts))
                emit_proj(*pending)
    nc.compile()
    return nc


def _prep_inputs(x, wq, gq, bq, wk, gk, bk, wv, gv, bv, wp, gp, bp):
    """Fold BN scales into weights; build the 8 per-core input maps."""
    rs = np.float32(1.0 / np.sqrt(np.float32(1.0) + np.float32(EPS)))
    sq = (gq * rs).astype(np.float32)
    sk = (gk * rs).astype(np.float32)
    sv = (gv * rs).astype(np.float32)
    sp = (gp * rs).astype(np.float32)
    wq_f = (wq * sq[:, None]).astype(np.float16)
    wk_f = (wk * sk[:, None]).astype(np.float16)
    wv_f = (wv * sv[:, None]).astype(np.float16)
    wp_f = (wp * sp[:, None]).astype(np.float16)

    xf = np.ascontiguousarray(x.reshape(B, C, N).astype(np.float16))
    consts = np.zeros((128, 128 + J), dtype=np.float32)
    consts[:, 0:128] = 1.0
    import ml_dtypes
    ones_bf = np.ones((128, 1), dtype=ml_dtypes.bfloat16)
    in_maps = []
    for core in range(8):
        b, g = core // 2, core % 2
        qs = slice(128 * g, 128 * (g + 1))       # q/k rows for this head group
        vs = slice(512 * g, 512 * (g + 1))       # v rows / p cols for this group
        in_maps.append({
            "x_in": xf[b],
            "wqt": np.ascontiguousarray(wq_f[qs, :].T),
            "wkt": np.ascontiguousarray(wk_f[qs, :].T),
            "wvt": np.ascontiguousarray(wv_f[vs, :].T),
            "wpt": np.ascontiguousarray(wp_f[:, vs].T),
            "bq": np.ascontiguousarray(bq[qs].astype(np.float32)[:, None]),
            "bk": np.ascontiguousarray(bk[qs].astype(np.float32)[:, None]),
            "bv": np.ascontiguousarray(bv[vs].astype(np.float32)[:, None]),
            "consts": consts,
            "ones_bf": ones_bf,
        })
    return in_maps


def kernel(**inputs):
    if "nc" not in _CACHE:
        _CACHE["nc"] = _build_program()
    nc = _CACHE["nc"]

    in_maps = _prep_inputs(**{k: np.asarray(v) for k, v in inputs.items()})
    res = run_bass_kernel_spmd(nc, in_maps, list(range(8)))
    _CACHE["last_results"] = res

    bp = np.asarray(inputs["bp"]).astype(np.float32)
    out = np.empty((B, C, H, W), dtype=np.float32)
    for b in range(B):
        acc = res.results[2 * b]["outp"] + res.results[2 * b + 1]["outp"]
        acc = acc + bp[:, None]
        out[b] = acc.reshape(C, H, W)
    return out


# revision 26
# speedup vs baseline: 1.4104x; 1.0671x over previous
"""Trainium2 Bass kernel for the BN-attention module (nn_Attention).

Full inputs -> full output. Sharding: 8 cores = (batch b in 0..3) x
(head-group g in 0..1, 4 heads each). Each core computes its batch's
4-head attention and a partial output projection; the host sums the two
head-group partials per batch and adds the projection BN bias.

Numerics: BN scales are folded into the weights on the host. The Q/K
path (x, wq, wk, q, k) runs in fp16 (11-bit mantissa; logit error
~4e-3 absolute). The V/P paths run 16-bit as well: vT and exp(S^T) are
bf16 (exp needs bf16 range, up to e^~25); relu output and wp are fp16.
Softmax skips the max-subtraction (logits are O(25), safe in fp32
accumulation); both attn@V and the softmax denominators consume the
same bf16 exp values so their rounding largely cancels in the ratio.

Layout: attention is computed transposed, S^T = K^T Q with keys (m) on
partitions, so attn@V needs no transposes at all: V is produced
directly as vT[n,d] by the projection. Denominators are column sums:
exp tiles are pair-summed on the VectorE (bf16 2x) into a 3-level
tree (9 -> 4+t8 -> 2+t8), then 3 ones-matmuls accumulate the column
sums on the TensorE. The finish phase (reciprocal, broadcast,
normalize, bias+relu) runs chunk-wide: one reciprocal per [128, 1024]
chunk, relu+bias as a dual-op tensor_scalar on the VectorE (fp16 out),
keeping the ScalarE exp-only.
"""

import numpy as np

import concourse.bacc as bacc
import concourse.mybir as mybir
import concourse.tile as tile
from concourse.bass_utils import run_bass_kernel_spmd

# Problem dims (hardcoded per the spec)
B, C, H, W = 4, 256, 48, 48
N = H * W            # 2304
KD, NH, AR = 32, 8, 4
D = AR * KD          # 128 value dims per head
NHKD = NH * KD       # 256
DH = NH * D          # 1024
EPS = 1e-5

NHG = 4              # heads per core
J = 256              # (legacy) consts tile width
MT = 128             # m-tile (key tile)
NMT = N // MT        # 18

F32 = mybir.dt.float32
F32R = mybir.dt.float32r
BF16 = mybir.dt.bfloat16
FP16 = mybir.dt.float16
AF = mybir.ActivationFunctionType
OP = mybir.AluOpType

# chunks of 2304 by <=512 for the projection matmuls
CHUNKS_512 = [(off, min(512, N - off)) for off in range(0, N, 512)]

_CACHE = {}


def _build_program():
    nc = bacc.Bacc("TRN2", target_bir_lowering=False, debug=False)

    x_in = nc.dram_tensor("x_in", [C, N], FP16, kind="ExternalInput")
    wqt_d = nc.dram_tensor("wqt", [C, 128], FP16, kind="ExternalInput")
    wkt_d = nc.dram_tensor("wkt", [C, 128], FP16, kind="ExternalInput")
    wvt_d = nc.dram_tensor("wvt", [C, 512], FP16, kind="ExternalInput")
    wpt_d = nc.dram_tensor("wpt", [512, C], FP16, kind="ExternalInput")
    bq_d = nc.dram_tensor("bq", [128, 1], F32, kind="ExternalInput")
    bk_d = nc.dram_tensor("bk", [128, 1], F32, kind="ExternalInput")
    bv_d = nc.dram_tensor("bv", [512, 1], F32, kind="ExternalInput")
    consts_d = nc.dram_tensor("consts", [128, 128 + J], F32R, kind="ExternalInput")
    ones_bf_d = nc.dram_tensor("ones_bf", [128, 1], BF16, kind="ExternalInput")
    out_d = nc.dram_tensor("outp", [C, N], F32, kind="ExternalOutput")

    with tile.TileContext(nc) as tc:
        with nc.allow_low_precision(reason="16-bit matmul rounding is intentional"), \
             tc.tile_pool(name="const", bufs=1) as constp, \
             tc.tile_pool(name="qk", bufs=1) as qkp, \
             tc.tile_pool(name="vt", bufs=1) as vtp, \
             tc.tile_pool(name="pexp", bufs=1) as pexpp, \
             tc.tile_pool(name="rp", bufs=1) as rp, \
             tc.tile_pool(name="work", bufs=2) as workp:

            # ---------- constants / inputs ----------
            # DMA order matters: the q/k weights + biases + x chunk 0 go
            # first so phase 1 starts as early as possible.
            wqt, wkt, wvt = [], [], []
            for c2 in range(2):
                sl = slice(128 * c2, 128 * (c2 + 1))
                t = constp.tile([128, 128], FP16, name=f"wqt{c2}", tag=f"wqt{c2}")
                nc.sync.dma_start(t[:], wqt_d.ap()[sl, :])
                wqt.append(t)
                t = constp.tile([128, 128], FP16, name=f"wkt{c2}", tag=f"wkt{c2}")
                nc.sync.dma_start(t[:], wkt_d.ap()[sl, :])
                wkt.append(t)
            bq_t = constp.tile([128, 1], F32, name="bq_t", tag="bq_t")
            nc.scalar.dma_start(bq_t[:], bq_d.ap())
            bk_t = constp.tile([128, 1], F32, name="bk_t", tag="bk_t")
            nc.scalar.dma_start(bk_t[:], bk_d.ap())
            xf = [constp.tile([128, N], FP16, name=f"xf{c2}", tag=f"xf{c2}")
                  for c2 in range(2)]
            for off, w in CHUNKS_512:
                nc.sync.dma_start(xf[0][:, off:off + w],
                                  x_in.ap()[0:128, off:off + w])
                nc.scalar.dma_start(xf[1][:, off:off + w],
                                    x_in.ap()[128:256, off:off + w])
                if off == 0:
                    for c2 in range(2):
                        t = constp.tile([128, 512], FP16, name=f"wvt{c2}",
                                        tag=f"wvt{c2}")
                        eng = nc.sync if c2 == 0 else nc.scalar
                        eng.dma_start(t[:], wvt_d.ap()[
                            128 * c2:128 * (c2 + 1), :])
                        wvt.append(t)
            wpt = []
            for h in range(NHG):
                t = constp.tile([128, C], FP16, name=f"wpt{h}", tag=f"wpt{h}")
                eng = nc.sync if h % 2 == 0 else nc.scalar
                eng.dma_start(t[:], wpt_d.ap()[128 * h:128 * (h + 1), :])
                wpt.append(t)
            bv_t = []
            for h in range(NHG):
                t = constp.tile([128, 1], F32, name=f"bv{h}", tag=f"bv{h}")
                nc.sync.dma_start(t[:], bv_d.ap()[128 * h:128 * (h + 1), :])
                bv_t.append(t)
            # constants via DMA (memset can't produce float32r)
            consts_t = constp.tile([128, 128 + J], F32R, name="consts_t",
                                   tag="consts_t")
            nc.sync.dma_start(consts_t[:], consts_d.ap())
            ones_all = consts_t[:, 0:128]     # [128,128] of 1.0
            ones_bf = constp.tile([128, 1], BF16, name="ones_bf", tag="ones_bf")
            nc.sync.dma_start(ones_bf[:], ones_bf_d.ap())

            q_all = qkp.tile([128, N], FP16, name="q_all", tag="q_all")
            k_all = qkp.tile([128, N], FP16, name="k_all", tag="k_all")
            vt_all = vtp.tile([128, NMT * 512], BF16, name="vt_all", tag="vt_all")

            # ---------- phase 1: q chunk 0 + all of k ----------
            # Just enough to start chunk 0's attention: q chunks 1-4 and all
            # v projections are deferred into chunk 0's loop below so the
            # ScalarE starts exp work as early as possible.
            with tc.tile_pool(name="p1", bufs=4, space="PSUM") as p1:
                ps = p1.tile([128, 512], F32, name="qproj", tag="p1")
                for c2 in range(2):
                    nc.tensor.matmul(ps[:], wqt[c2][:], xf[c2][:, 0:512],
                                     start=(c2 == 0), stop=(c2 == 1))
                nc.vector.tensor_scalar_add(q_all[:, 0:512], ps[:], bq_t[:])
                for off, w in CHUNKS_512:
                    ps = p1.tile([128, 512], F32, name="kproj", tag="p1")
                    for c2 in range(2):
                        nc.tensor.matmul(ps[:, :w], wkt[c2][:],
                                         xf[c2][:, off:off + w],
                                         start=(c2 == 0), stop=(c2 == 1))
                    nc.vector.tensor_scalar_add(k_all[:, off:off + w],
                                                ps[:, :w], bk_t[:])

            # ---------- phase 2: attention + output projection ----------
            # n-chunks of width 512 (last 256); heads processed two at a
            # time (half-chunks). Per m-tile step, the two heads' S^T
            # matmuls land in the two banks of one [128,1024] psum tile
            # (alternating between two such tiles) and a single exp call
            # evicts both to a bf16 pexp tile. PV runs one step behind the
            # QKs so the PE never waits on ACT; consecutive pexp steps are
            # pair-summed on DVE (bf16 2x) into resident t-tiles, which a
            # 2-more-level DVE tree folds to 3 tiles so the denominator
            # needs only 3 ones-matmuls. The finish phase (reciprocal,
            # broadcast, normalize, fp16 relu+bias on DVE) runs chunk-wide
            # in two dedicated psum banks, overlapping the next
            # half-chunk's loop. The output projection needs all four
            # heads, so it runs after the second half-chunk.
            NP = NMT // 2
            G2 = ((0, 1), (2, 3))
            JCHUNKS = [(0, 512, G2), (512, 512, G2), (1024, 512, G2),
                       (1536, 512, G2), (2048, 256, G2)]
            with tc.tile_pool(name="stp", bufs=1, space="PSUM") as stp, \
                 tc.tile_pool(name="xxp", bufs=1, space="PSUM") as xxp, \
                 tc.tile_pool(name="finp", bufs=1, space="PSUM") as finp:
                def emit_proj(joff_p, JW_p, r_p):
                    # output projection over all four heads of a finished
                    # chunk (deferred into the next chunk's loop so the PE
                    # never stalls on the finish chain at chunk boundaries)
                    for ct in range(2):
                        op_ps = finp.tile([128, 512], F32, name="op_ps",
                                          tag=f"f{ct}")
                        for h in range(NHG):
                            nc.tensor.matmul(
                                op_ps[:, 0:JW_p],
                                wpt[h][:, 128 * ct:128 * (ct + 1)],
                                r_p[h][:, 0:JW_p],
                                start=(h == 0), stop=(h == NHG - 1))
                        o_sb = workp.tile([128, 512], F32, name="o_sb",
                                          tag="o_sb")
                        nc.vector.tensor_copy(o_sb[:, 0:JW_p],
                                              op_ps[:, 0:JW_p])
                        nc.sync.dma_start(
                            out_d.ap()[128 * ct:128 * (ct + 1),
                                       joff_p:joff_p + JW_p],
                            o_sb[:, 0:JW_p])

                # q-projection schedule inside chunk 0 (mt -> q chunk)
                QSCHED = {4: 1, 7: 2, 10: 3, 13: 4}

                pending = None     # (joff, JW, r_ts) of the previous chunk
                fin_prev = None    # previous pair's deferred finish closure
                pair_tasks = []
                for ci, (joff, JW, groups) in enumerate(JCHUNKS):
                    for gi, grp in enumerate(groups):
                        pair_tasks.append((ci, joff, JW, gi, grp,
                                           gi == len(groups) - 1))
                r_ts = None
                for ci, joff, JW, gi, grp, last_in_chunk in pair_tasks:
                    if True:
                        if gi == 0:
                            r_ts = [None] * NHG
                        G = len(grp)           # heads in this group
                        xxt = [xxp.tile([128, 512], F32, name=f"xx{j}",
                                        tag=f"xx{j}") for j in range(2)]
                        xx = {}
                        for i, h in enumerate(grp):
                            xx[h] = xxt[i][:, 0:JW]
                        pexp = [None] * NMT
                        tsum = [None] * NP
                        usum = [None] * 4
                        wsum = [None] * 2

                        def emit_qk_exp(mt):
                            # the G heads' matmuls run concurrently
                            # (different PE row groups); each head's slice
                            # stays within a psum bank.
                            st = stp.tile([128, 1024], F32, name="st",
                                          tag=f"st{mt % 2}")
                            moff = 128 * mt
                            for i, h in enumerate(grp):
                                nc.tensor.matmul(
                                    st[:, 512 * i:512 * i + JW],
                                    k_all[32 * h:32 * (h + 1), moff:moff + 128],
                                    q_all[32 * h:32 * (h + 1), joff:joff + JW],
                                    start=True, stop=True,
                                    tile_position=(32 * h, 0))
                            pe = pexpp.tile([128, 1024], BF16, name="pe",
                                            tag="pe", bufs=4)
                            if JW == 512:
                                nc.scalar.activation(pe[:, 0:1024],
                                                     st[:, 0:1024], AF.Exp)
                            else:
                                st_v = st.rearrange("p (a b) -> p a b",
                                                    b=512)[:, :, 0:JW]
                                pe_v = pe.rearrange("p (a b) -> p a b",
                                                    b=512)[:, :, 0:JW]
                                nc.scalar.activation(pe_v, st_v, AF.Exp)
                            pexp[mt] = pe

                        def emit_pv(mt, first=False):
                            # PV(1) executes first (psum reset); PV(0) joins
                            # late so the new pair's xx reset never waits on
                            # the previous pair's finish chain reading xx.
                            pe = pexp[mt]
                            for i, h in enumerate(grp):
                                nc.tensor.matmul(
                                    xx[h],
                                    vt_all[:, 512 * mt + 128 * h:
                                           512 * mt + 128 * (h + 1)],
                                    pe[:, 512 * i:512 * i + JW],
                                    start=first, stop=(mt == NMT - 1),
                                    skip_group_check=True)

                        def emit_tree(k):
                            # level-2/3/4 folds as soon as inputs are ready;
                            # W = t0+..+t7 is complete by mt 15 so only t8
                            # (the last exp pair) is left for the finish.
                            if k % 2 == 1 and k < 8:
                                u = k // 2
                                t = workp.tile([128, 1024], BF16,
                                               name=f"u{u}", tag=f"u{u}",
                                               bufs=1)
                                nc.vector.tensor_tensor(
                                    t[:], tsum[k - 1][:], tsum[k][:], OP.add)
                                usum[u] = t
                                if u % 2 == 1:
                                    w_ = u // 2
                                    t2 = workp.tile([128, 1024], BF16,
                                                    name=f"w{w_}",
                                                    tag=f"w{w_}", bufs=1)
                                    nc.vector.tensor_tensor(
                                        t2[:], usum[u - 1][:], usum[u][:],
                                        OP.add)
                                    wsum[w_] = t2
                                    if w_ == 1:
                                        t3 = workp.tile([128, 1024], BF16,
                                                        name="wall",
                                                        tag="wall", bufs=1)
                                        nc.vector.tensor_tensor(
                                            t3[:], wsum[0][:], wsum[1][:],
                                            OP.add)
                                        wsum.append(t3)

                        sums_hs = [None, None]
                        for mt in range(NMT):
                            if ci == 0 and gi == 0:
                                # v projection for m-tile mt, one step ahead
                                # of its PV consumer; evicted on ScalarE
                                ps_v = finp.tile([128, 512], F32, name="vps",
                                                 tag=f"f{mt % 2}")
                                for c2 in range(2):
                                    nc.tensor.matmul(
                                        ps_v[:],
                                        xf[c2][:, 128 * mt:128 * (mt + 1)],
                                        wvt[c2][:],
                                        start=(c2 == 0), stop=(c2 == 1))
                                nc.scalar.copy(
                                    vt_all[:, 512 * mt:512 * (mt + 1)],
                                    ps_v[:])
                                if mt in QSCHED:
                                    qc = QSCHED[mt]
                                    qo = 512 * qc
                                    qw = min(512, N - qo)
                                    ps_q = finp.tile([128, 512], F32,
                                                     name="qps",
                                                     tag=f"f{(mt + 1) % 2}")
                                    for c2 in range(2):
                                        nc.tensor.matmul(
                                            ps_q[:, 0:qw], wqt[c2][:],
                                            xf[c2][:, qo:qo + qw],
                                            start=(c2 == 0), stop=(c2 == 1))
                                    nc.vector.tensor_scalar_add(
                                        q_all[:, qo:qo + qw],
                                        ps_q[:, 0:qw], bq_t[:])
                            emit_qk_exp(mt)
                            if mt == 1 and fin_prev is not None:
                                fin_prev()
                                fin_prev = None
                            if mt == 2:
                                emit_pv(1, first=True)
                            elif mt == 3:
                                emit_pv(2)
                                emit_pv(0)
                            elif mt >= 4:
                                emit_pv(mt - 1)
                            if mt == 3 and gi == 0 and pending is not None:
                                emit_proj(*pending)
                                pending = None
                            if mt == NMT - 1:
                                # W covers t0..t7 (32 of 36 m-tiles); start
                                # the denominator accumulation while the
                                # last exp is still in flight.
                                for i in range(G):
                                    sums_hs[i] = finp.tile(
                                        [1, 512], F32, name="sums_h",
                                        tag=f"f{i % 2}")
                                    nc.tensor.matmul(
                                        sums_hs[i][:, 0:JW], ones_bf[:],
                                        wsum[2][:, 512 * i:512 * i + JW],
                                        start=True, stop=False)
                            if mt % 2 == 1:
                                k = mt // 2
                                t = workp.tile([128, 1024], BF16,
                                               name=f"t{k}", tag=f"t{k}",
                                               bufs=1)
                                nc.vector.tensor_tensor(
                                    t[:], pexp[mt - 1][:], pexp[mt][:],
                                    OP.add)
                                tsum[k] = t
                                emit_tree(k)
                        emit_pv(NMT - 1)

                        def make_finish(grp=grp, xx=xx, tsum=tsum,
                                        sums_hs=sums_hs, r_out=r_ts, JW=JW):
                            def fin():
                                # finish: denominators, normalize, bias+relu.
                                # Only the t8 ones-matmul waits on the last
                                # exp; reciprocal runs on the [1,JW] sums row
                                # and GPSIMD broadcasts it across partitions.
                                for i, h in enumerate(grp):
                                    sums_h = sums_hs[i]
                                    nc.tensor.matmul(
                                        sums_h[:, 0:JW], ones_bf[:],
                                        tsum[8][:, 512 * i:512 * i + JW],
                                        start=False, stop=True)
                                    s_inv = workp.tile([1, 512], F32,
                                                       name="s_inv",
                                                       tag="s_inv")
                                    nc.vector.reciprocal_approx_fast(
                                        s_inv[:, 0:JW], sums_h[:, 0:JW])
                                    inv_bc = workp.tile([128, 512], F32,
                                                        name="inv_bc",
                                                        tag="inv_bc")
                                    nc.gpsimd.partition_broadcast(
                                        inv_bc[:, 0:JW], s_inv[:, 0:JW])
                                    t_h = workp.tile([128, 512], F32,
                                                     name="t_h", tag="t_h")
                                    nc.vector.tensor_tensor(
                                        t_h[:, 0:JW], xx[h],
                                        inv_bc[:, 0:JW], OP.mult)
                                    r_h = rp.tile([128, 512], FP16,
                                                  name=f"r{h}", tag=f"r{h}")
                                    nc.vector.tensor_scalar(
                                        r_h[:, 0:JW], t_h[:, 0:JW],
                                        bv_t[h][:], 0.0, OP.add, OP.max)
                                    r_out[h] = r_h
                            return fin

                        fin_prev = make_finish()
                        if last_in_chunk:
                            pending = (joff, JW, r_ts)
                fin_prev()
                emit_proj(*pending)
    nc.compile()
    return nc


def _prep_inputs(x, wq, gq, bq, wk, gk, bk, wv, gv, bv, wp, gp, bp):
    """Fold BN scales into weights; build the 8 per-core input maps."""
    rs = np.float32(1.0 / np.sqrt(np.float32(1.0) + np.float32(EPS)))
    sq = (gq * rs).astype(np.float32)
    sk = (gk * rs).astype(np.float32)
    sv = (gv * rs).astype(np.float32)
    sp = (gp * rs).astype(np.float32)
    wq_f = (wq * sq[:, None]).astype(np.float16)
    wk_f = (wk * sk[:, None]).astype(np.float16)
    wv_f = (wv * sv[:, None]).astype(np.float16)
    wp_f = (wp * sp[:, None]).astype(np.float16)

    xf = np.ascontiguousarray(x.reshape(B, C, N).astype(np.float16))
    consts = np.zeros((128, 128 + J), dtype=np.float32)
    consts[:, 0:128] = 1.0
    import ml_dtypes
    ones_bf = np.ones((128, 1), dtype=ml_dtypes.bfloat16)
    in_maps = []
    for core in range(8):
        b, g = core // 2, core % 2
        qs = slice(128 * g, 128 * (g + 1))       # q/k rows for this head group
        vs = slice(512 * g, 512 * (g + 1))       # v rows / p cols for this group
        in_maps.append({
            "x_in": xf[b],
            "wqt": np.ascontiguousarray(wq_f[qs, :].T),
            "wkt": np.ascontiguousarray(wk_f[qs, :].T),
            "wvt": np.ascontiguousarray(wv_f[vs, :].T),
            "wpt": np.ascontiguousarray(wp_f[:, vs].T),
            "bq": np.ascontiguousarray(bq[qs].astype(np.float32)[:, None]),
            "bk": np.ascontiguousarray(bk[qs].astype(np.float32)[:, None]),
            "bv": np.ascontiguousarray(bv[vs].astype(np.float32)[:, None]),
            "consts": consts,
            "ones_bf": ones_bf,
        })
    return in_maps


def kernel(**inputs):
    if "nc" not in _CACHE:
        _CACHE["nc"] = _build_program()
    nc = _CACHE["nc"]

    in_maps = _prep_inputs(**{k: np.asarray(v) for k, v in inputs.items()})
    res = run_bass_kernel_spmd(nc, in_maps, list(range(8)))
    _CACHE["last_results"] = res

    bp = np.asarray(inputs["bp"]).astype(np.float32)
    out = np.empty((B, C, H, W), dtype=np.float32)
    for b in range(B):
        acc = res.results[2 * b]["outp"] + res.results[2 * b + 1]["outp"]
        acc = acc + bp[:, None]
        out[b] = acc.reshape(C, H, W)
    return out
